# revision 22
# baseline (speedup 1.0000x reference)
"""Trainium2 Bass kernel for nn_ACTGraphLayer (gnn_message_passing).

Data-parallel over B=256 rollout threads: 32 rows per NeuronCore x 8 cores.

Per core:
  - father_flat shard [32, 65536] is viewed as [2048, 1024] rows; only every
    16th float (slot 0 of each action block) is nonzero: adj[b,j,p]*fw[p].
    We keep zero-initialized [128, 2048] SBUF tiles, overwrite the stride-16
    slot-0 lanes with strided vector copies, and stream 8 contiguous 1 MB
    stores alternating across the two HWDGE rings (sync + scalar).
  - logits[b,a] = x@Wd.T + sum_f adj[b,f]*fw[f%64]*Wf[f,a] + bias, computed
    on the TensorEngine (PE transposes of adj/x feed 34 accumulating
    matmuls).  Only the structurally-live columns of W are shipped to the
    device (pure host-side slicing/relayout of the weight tensor - no host
    arithmetic).  Softmax/argmax run on-device (max8/max_index + Exp/Ln).
"""

import os
import sys

import numpy as np

for _p in ("/opt/trn_rl_repo", "/root/.axon_site/_ro/trn_rl_repo"):
    if os.path.isdir(_p) and _p not in sys.path:
        sys.path.insert(0, _p)

B, N, A, D = 256, 64, 16, 256
M = 8                      # cores
BL = B // M                # 32 rows per core
NG = N * N                 # 4096 father groups per row
R = BL * N                 # 2048 output rows of 1024 floats per core
KCH = NG // 128            # 32 matmul K-chunks over the father axis
DCH = D // 128             # 2 matmul K-chunks over the x axis
NEG_INF = -1e10
TCH = R // 128             # 16 row-groups per partition
SUB = 2                    # row-groups per store (1 MB stores)
NST = TCH // SUB           # 8 stores
NEXP = 4                   # expansion tile ring depth

_CACHE = {}


def _build_nc():
    import concourse.bass as bass
    import concourse.mybir as mybir
    import concourse.tile as tile
    from concourse.masks import make_identity
    from contextlib import ExitStack

    f32 = mybir.dt.float32
    i32 = mybir.dt.int32
    u32 = mybir.dt.uint32

    nc = bass.Bass()
    adj_d = nc.declare_dram_parameter("adj", [R, N], i32, isOutput=False)
    x_d = nc.declare_dram_parameter("x", [BL, D], f32, isOutput=False)
    av_d = nc.declare_dram_parameter("avail", [BL, A], f32, isOutput=False)
    fw_d = nc.declare_dram_parameter("fw", [1, N], f32, isOutput=False)
    fwc_d = nc.declare_dram_parameter("fw_col", [128, 1], f32, isOutput=False)
    wft_d = nc.declare_dram_parameter("wft", [128, KCH * A], f32, isOutput=False)
    wdt_d = nc.declare_dram_parameter("wdt", [128, DCH * A], f32, isOutput=False)
    b_d = nc.declare_dram_parameter("bias", [1, A], f32, isOutput=False)
    fat_d = nc.declare_dram_parameter("father", [R, N * A], f32, isOutput=True)
    act_d = nc.declare_dram_parameter("actions", [BL, 1], i32, isOutput=True)
    alp_d = nc.declare_dram_parameter("alp", [BL, 1], f32, isOutput=True)

    with ExitStack() as ctx:
        tc = ctx.enter_context(tile.TileContext(nc))
        singles = ctx.enter_context(tc.tile_pool(name="singles", bufs=1))
        expp = ctx.enter_context(tc.tile_pool(name="expp", bufs=1))
        psump = ctx.enter_context(tc.tile_pool(name="psump", bufs=6, space="PSUM"))
        lgp = ctx.enter_context(tc.tile_pool(name="lgp", bufs=1, space="PSUM"))

        # ---- early loads ----
        # sync ring: adj in expansion layout first (feeds the store stream).
        # Partition p holds output rows 16p..16p+15, so the load is fully
        # contiguous per partition (4 KB descriptors) and each store writes
        # 128 contiguous 4 KB runs.
        aj_all = singles.tile([128, TCH, N], i32)
        aj_src = adj_d[:].rearrange("(p t) n -> p t n", p=128)
        PIECES = [(0, 2), (2, 6), (8, 4), (12, 4)]  # (start group, n groups)
        for c, (s0, ln) in enumerate(PIECES):
            eng = nc.sync if c % 2 == 0 else nc.scalar
            eng.dma_start(
                out=aj_all[:, s0:s0 + ln, :], in_=aj_src[:, s0:s0 + ln, :]
            )
        # weight/small loads are emitted later (interleaved into the scalar
        # ring behind its first stores); declare tiles here
        wf_raw = singles.tile([128, KCH * A], f32)
        fwcol = singles.tile([128, 1], f32)
        xb = singles.tile([BL, D], f32)
        wd_sb = singles.tile([128, DCH * A], f32)
        av_sb = singles.tile([BL, A], f32)

        # POOL: broadcasts + identity + PE-path cast-load + half the memsets
        fwrow = singles.tile([128, N], f32)
        nc.gpsimd.dma_start(out=fwrow[:], in_=fw_d[:].to_broadcast((128, N)))
        ident = singles.tile([128, 128], f32)
        make_identity(nc, ident[:])
        exp_tiles = [
            expp.tile([128, SUB, N * A], f32, tag=f"exp{i}", name=f"exp{i}")
            for i in range(NEXP)
        ]
        # memsets split across DVE (fills its pre-data idle) and POOL
        for i, t in enumerate(exp_tiles):
            (nc.vector if i % 2 == 0 else nc.gpsimd).memset(t[:], 0.0)

        ab = singles.tile([BL, NG], f32)
        nc.gpsimd.dma_start(  # SWDGE cast-DMA: int32 adj rows -> f32
            out=ab[:], in_=adj_d[:].rearrange("(b n) p -> b (n p)", b=BL)
        )
        b_sb = singles.tile([BL, A], f32)
        nc.gpsimd.dma_start(out=b_sb[:], in_=b_d[:].to_broadcast((BL, A)))

        # ---- expansion stream: ajf piece -> copies -> stores, interleaved ----
        ajf = singles.tile([128, TCH, N], f32)
        fw3 = fwrow[:].rearrange("p (o n) -> p o n", o=1)
        fat_v = fat_d[:].rearrange("(p u c) n -> u p c n", u=NST, c=SUB)
        piece_starts = {s0: ln for s0, ln in PIECES}
        wloads = [(wf_raw, wft_d), (fwcol, fwc_d), (xb, x_d), (wd_sb, wdt_d),
                  (av_sb, av_d)]
        for u in range(NST):
            ex = exp_tiles[u % NEXP]
            exv = ex[:].rearrange("p c (g a) -> p c g a", a=A)
            for i in range(SUB):
                t = SUB * u + i
                if t in piece_starts:
                    ln = piece_starts[t]
                    nc.vector.tensor_tensor(
                        out=ajf[:, t:t + ln, :],
                        in0=aj_all[:, t:t + ln, :],
                        in1=fw3.to_broadcast((128, ln, N)),
                        op=mybir.AluOpType.mult,
                    )
                nc.vector.tensor_copy(
                    out=exv[:, i, :, 0:1],
                    in_=ajf[:, t, :].rearrange("p (g o) -> p g o", o=1),
                )
            eng = nc.sync if u % 2 == 0 else nc.scalar
            eng.dma_start(out=fat_v[u], in_=ex[:])
            # slot the weight/small loads onto the scalar ring behind its
            # first stores (they are needed only by the PE/epilogue phase)
            if u == 1:
                for tile_, src_ in wloads[:2]:
                    nc.scalar.dma_start(out=tile_[:], in_=src_[:])
            if u == 3:
                for tile_, src_ in wloads[2:]:
                    nc.scalar.dma_start(out=tile_[:], in_=src_[:])

        # fold father_weights into the father weight columns (per partition k,
        # the factor is fw[k % 64], precomputed on host as fw_col)
        wf_sb = singles.tile([128, KCH * A], f32)
        nc.vector.tensor_tensor(
            out=wf_sb[:], in0=wf_raw[:],
            in1=fwcol[:].to_broadcast((128, KCH * A)),
            op=mybir.AluOpType.mult,
        )

        # ---- PE transposes: 4 per PSUM tile, one DVE copy per group ----
        fsb = singles.tile([128, KCH * BL], f32)
        xt = singles.tile([128, DCH * BL], f32)
        srcs = [("f", c) for c in range(KCH)] + [("x", c) for c in range(DCH)]
        for g in range(0, len(srcs), 4):
            grp = srcs[g:g + 4]
            pt = psump.tile([128, 4, BL], f32, tag="pt", name="pt")
            for gi, (kind, c) in enumerate(grp):
                src = ab if kind == "f" else xb
                nc.tensor.transpose(
                    pt[:, gi, :], src[:, 128 * c:128 * (c + 1)], ident[:BL, :BL]
                )
            for gi, (kind, c) in enumerate(grp):
                dst = fsb if kind == "f" else xt
                nc.vector.tensor_copy(
                    out=dst[:, c * BL:(c + 1) * BL], in_=pt[:, gi, :]
                )

        lg = lgp.tile([BL, A], f32)
        for c in range(KCH):
            nc.tensor.matmul(
                lg[:],
                lhsT=fsb[:, BL * c:BL * (c + 1)],
                rhs=wf_sb[:, A * c:A * (c + 1)],
                start=(c == 0),
                stop=False,
            )
        for c in range(DCH):
            nc.tensor.matmul(
                lg[:],
                lhsT=xt[:, BL * c:BL * (c + 1)],
                rhs=wd_sb[:, A * c:A * (c + 1)],
                start=False,
                stop=(c == DCH - 1),
            )

        # ---- mask + log-softmax + argmax ----
        neg_sb = singles.tile([BL, A], f32)
        nc.vector.memset(neg_sb[:], NEG_INF)
        av_m = singles.tile([BL, A], mybir.dt.int8)
        nc.vector.tensor_scalar(
            out=av_m[:], in0=av_sb[:], scalar1=0.0, scalar2=None,
            op0=mybir.AluOpType.is_gt,
        )
        lg_sb = singles.tile([BL, A], f32)
        nc.vector.tensor_tensor(
            out=lg_sb[:], in0=lg[:], in1=b_sb[:], op=mybir.AluOpType.add
        )
        ml = singles.tile([BL, A], f32)
        nc.vector.tensor_copy(out=ml[:], in_=neg_sb[:])
        nc.vector.copy_predicated(out=ml[:], mask=av_m[:], data=lg_sb[:])

        m8 = singles.tile([BL, 8], f32)
        i8u = singles.tile([BL, 8], u32)
        nc.vector.max(m8[:], ml[:])
        nc.vector.max_index(i8u[:], m8[:], ml[:])

        sh = singles.tile([BL, A], f32)
        nc.vector.tensor_scalar(
            out=sh[:], in0=ml[:], scalar1=m8[:, 0:1], scalar2=None,
            op0=mybir.AluOpType.subtract,
        )
        et = singles.tile([BL, A], f32)
        ssum = singles.tile([BL, 1], f32)
        nc.scalar.activation(
            out=et[:], in_=sh[:], func=mybir.ActivationFunctionType.Exp,
            accum_out=ssum[:],
        )
        lns = singles.tile([BL, 1], f32)
        nc.scalar.activation(
            out=lns[:], in_=ssum[:], func=mybir.ActivationFunctionType.Ln
        )
        alp_sb = singles.tile([BL, 1], f32)
        nc.vector.tensor_scalar_mul(out=alp_sb[:], in0=lns[:], scalar1=-1.0)
        act_sb = singles.tile([BL, 1], i32)
        nc.vector.tensor_copy(out=act_sb[:], in_=i8u[:, 0:1])

        nc.sync.dma_start(out=act_d[:], in_=act_sb[:])
        nc.sync.dma_start(out=alp_d[:], in_=alp_sb[:])

    _legalize_multi_waits(nc, mybir)
    return nc


def _legalize_multi_waits(nc, mybir):
    """This toolchain's walrus accepts at most one embedded sync-wait per
    compute/DMA instruction (two for EventSemaphore).  Tile's sem assignment
    can emit more; spill the extras onto same-engine NoOp carriers inserted
    immediately before the instruction."""
    n = 0
    for fn in nc.m.functions:
        for blk in fn.blocks:
            insts = blk.instructions
            i = 0
            while i < len(insts):
                inst = insts[i]
                si = inst.sync_info
                cap = 2 if isinstance(inst, mybir.InstEventSemaphore) else 1
                if si is not None and len(si.on_wait) > cap:
                    waits = list(si.on_wait)
                    keep, extra = waits[-cap:], waits[:-cap]
                    inst.sync_info = mybir.SyncInfo(
                        on_wait=keep, on_update=list(si.on_update)
                    )
                    for w in extra:
                        n += 1
                        nop = mybir.InstNoOp(name=f"WSPLIT-{n}", ins=[], outs=[])
                        nop.engine = inst.engine
                        nop.sync_info = mybir.SyncInfo(on_wait=[w], on_update=[])
                        insts.insert(i, nop)
                        i += 1
                i += 1
    return n


def _get_nc():
    if "nc" not in _CACHE:
        _CACHE["nc"] = _build_nc()
    return _CACHE["nc"]


def _make_in_maps(x, adj, available_actions, father_weights, W, b):
    x = np.ascontiguousarray(x, dtype=np.float32)
    adj = np.ascontiguousarray(adj, dtype=np.int32)
    av = np.ascontiguousarray(available_actions, dtype=np.float32)
    fw = np.ascontiguousarray(father_weights, dtype=np.float32)
    W = np.ascontiguousarray(W, dtype=np.float32)
    b = np.ascontiguousarray(b, dtype=np.float32)

    # host-side relayout of the structurally-live weight columns (no math)
    wdt = np.ascontiguousarray(
        W[:, :D].T.reshape(DCH, 128, A).transpose(1, 0, 2).reshape(128, DCH * A)
    )
    wft = np.ascontiguousarray(
        W[:, D::A].T.reshape(KCH, 128, A).transpose(1, 0, 2).reshape(128, KCH * A)
    )
    fw_col = np.ascontiguousarray(np.concatenate([fw, fw])[:, None])
    fw_row = np.ascontiguousarray(fw[None, :])
    bias = np.ascontiguousarray(b[None, :])

    in_maps = []
    for i in range(M):
        sl = slice(BL * i, BL * (i + 1))
        in_maps.append({
            "adj": np.ascontiguousarray(adj[sl].reshape(R, N)),
            "x": np.ascontiguousarray(x[sl]),
            "avail": np.ascontiguousarray(av[sl]),
            "fw": fw_row,
            "fw_col": fw_col,
            "wft": wft,
            "wdt": wdt,
            "bias": bias,
        })
    return in_maps


def _assemble(results):
    actions = np.concatenate([r["actions"] for r in results], axis=0).astype(np.int32)
    alp = np.concatenate([r["alp"] for r in results], axis=0).astype(np.float32)
    father = np.concatenate(
        [r["father"].reshape(BL, NG * A) for r in results], axis=0
    ).astype(np.float32)
    return actions, alp, father


def run_on_device(inputs, trace=False, **kw):
    """Compile+run the SPMD bass kernel; returns (outputs_tuple, BassKernelResults)."""
    from concourse.bass_utils import run_bass_kernel_spmd

    nc = _get_nc()
    in_maps = _make_in_maps(**inputs)
    res = run_bass_kernel_spmd(nc, in_maps, core_ids=list(range(M)), trace=trace, **kw)
    return _assemble(res.results), res


def kernel(**inputs):
    (actions, alp, father), _ = run_on_device(inputs, trace=False)
    return actions, alp, father


# revision 29
# speedup vs baseline: 1.0324x; 1.0324x over previous
"""Trainium2 Bass kernel for nn_ACTGraphLayer (gnn_message_passing).

Data-parallel over B=256 rollout threads: 32 rows per NeuronCore x 8 cores.

Per core:
  - father_flat shard [32, 65536] is viewed as [2048, 1024] rows; only every
    16th float (slot 0 of each action block) is nonzero: adj[b,j,p]*fw[p].
    Partition p owns output rows 16p..16p+15 so the adj load and all stores
    are contiguous per partition.  Zero-initialized [128, 1024] SBUF tiles
    get their stride-16 slot-0 lanes overwritten by strided vector copies,
    then stream out as 16 x 512 KB contiguous stores alternating across the
    two HWDGE rings (sync + scalar).
  - logits[b,a] = x@Wd.T + sum_f adj[b,f]*fw[f%64]*Wf[f,a] + bias, computed
    on the TensorEngine (PE transposes of adj/x feed 34 accumulating
    matmuls).  Only the structurally-live columns of W are shipped to the
    device (pure host-side slicing/relayout of the weight tensor - no host
    arithmetic).  Softmax/argmax run on-device (max8/max_index + Exp/Ln).
"""

import os
import sys

import numpy as np

for _p in ("/opt/trn_rl_repo", "/root/.axon_site/_ro/trn_rl_repo"):
    if os.path.isdir(_p) and _p not in sys.path:
        sys.path.insert(0, _p)

B, N, A, D = 256, 64, 16, 256
M = 8                      # cores
BL = B // M                # 32 rows per core
NG = N * N                 # 4096 father groups per row
R = BL * N                 # 2048 output rows of 1024 floats per core
KCH = NG // 128            # 32 matmul K-chunks over the father axis
DCH = D // 128             # 2 matmul K-chunks over the x axis
NEG_INF = -1e10
TCH = R // 128             # 16 row-groups per partition
SUB = 1                    # row-groups per store (512 KB stores)
NST = TCH // SUB           # 16 stores
NEXP = 10                  # expansion tile ring depth

_CACHE = {}


def _build_nc():
    import concourse.bass as bass
    import concourse.mybir as mybir
    import concourse.tile as tile
    from concourse.masks import make_identity
    from contextlib import ExitStack

    f32 = mybir.dt.float32
    i32 = mybir.dt.int32
    u32 = mybir.dt.uint32

    nc = bass.Bass()
    adj_d = nc.declare_dram_parameter("adj", [R, N], i32, isOutput=False)
    x_d = nc.declare_dram_parameter("x", [BL, D], f32, isOutput=False)
    av_d = nc.declare_dram_parameter("avail", [BL, A], f32, isOutput=False)
    fw_d = nc.declare_dram_parameter("fw", [1, N], f32, isOutput=False)
    fwc_d = nc.declare_dram_parameter("fw_col", [128, 1], f32, isOutput=False)
    wft_d = nc.declare_dram_parameter("wft", [128, KCH * A], f32, isOutput=False)
    wdt_d = nc.declare_dram_parameter("wdt", [128, DCH * A], f32, isOutput=False)
    b_d = nc.declare_dram_parameter("bias", [1, A], f32, isOutput=False)
    fat_d = nc.declare_dram_parameter("father", [R, N * A], f32, isOutput=True)
    act_d = nc.declare_dram_parameter("actions", [BL, 1], i32, isOutput=True)
    alp_d = nc.declare_dram_parameter("alp", [BL, 1], f32, isOutput=True)

    with ExitStack() as ctx:
        tc = ctx.enter_context(tile.TileContext(nc))
        singles = ctx.enter_context(tc.tile_pool(name="singles", bufs=1))
        expp = ctx.enter_context(tc.tile_pool(name="expp", bufs=1))
        psump = ctx.enter_context(tc.tile_pool(name="psump", bufs=6, space="PSUM"))
        lgp = ctx.enter_context(tc.tile_pool(name="lgp", bufs=1, space="PSUM"))

        # ---- early loads ----
        # sync ring: adj in expansion layout first (feeds the store stream).
        # Partition p holds output rows 16p..16p+15, so the load is fully
        # contiguous per partition (4 KB descriptors) and each store writes
        # 128 contiguous 4 KB runs.
        aj_all = singles.tile([128, TCH, N], i32)
        aj_src = adj_d[:].rearrange("(p t) n -> p t n", p=128)
        PIECES = [(0, 2), (2, 6), (8, 4), (12, 4)]  # (start group, n groups)
        for c, (s0, ln) in enumerate(PIECES):
            eng = nc.sync if c % 2 == 0 else nc.scalar
            eng.dma_start(
                out=aj_all[:, s0:s0 + ln, :], in_=aj_src[:, s0:s0 + ln, :]
            )
        # weight/small loads are emitted later (interleaved into the scalar
        # ring behind its first stores); declare tiles here
        wf_raw = singles.tile([128, KCH * A], f32)
        fwcol = singles.tile([128, 1], f32)
        xb = singles.tile([BL, D], f32)
        wd_sb = singles.tile([128, DCH * A], f32)
        av_sb = singles.tile([BL, A], f32)

        # POOL: broadcasts + identity + PE-path cast-load + half the memsets
        fwrow = singles.tile([128, N], f32)
        nc.gpsimd.dma_start(out=fwrow[:], in_=fw_d[:].to_broadcast((128, N)))
        ident = singles.tile([128, 128], f32)
        make_identity(nc, ident[:])
        exp_tiles = [
            expp.tile([128, SUB, N * A], f32, tag=f"exp{i}", name=f"exp{i}")
            for i in range(NEXP)
        ]
        # tiles are zeroed just-in-time inside the store loop (DVE/POOL split)

        ab = singles.tile([BL, NG], f32)
        nc.gpsimd.dma_start(  # SWDGE cast-DMA: int32 adj rows -> f32
            out=ab[:], in_=adj_d[:].rearrange("(b n) p -> b (n p)", b=BL)
        )
        b_sb = singles.tile([BL, A], f32)
        nc.gpsimd.dma_start(out=b_sb[:], in_=b_d[:].to_broadcast((BL, A)))

        # ---- expansion stream: ajf piece -> copies -> stores, interleaved ----
        ajf = singles.tile([128, TCH, N], f32)
        fw3 = fwrow[:].rearrange("p (o n) -> p o n", o=1)
        fat_v = fat_d[:].rearrange("(p u c) n -> u p c n", u=NST, c=SUB)
        piece_starts = {s0: ln for s0, ln in PIECES}
        wloads = [(wf_raw, wft_d), (fwcol, fwc_d), (xb, x_d), (wd_sb, wdt_d),
                  (av_sb, av_d)]
        for u in range(NST):
            ex = exp_tiles[u % NEXP]
            if u < NEXP:
                (nc.vector if u % 2 == 0 else nc.gpsimd).memset(ex[:], 0.0)
            exv = ex[:].rearrange("p c (g a) -> p c g a", a=A)
            for i in range(SUB):
                t = SUB * u + i
                if t in piece_starts:
                    ln = piece_starts[t]
                    nc.vector.tensor_tensor(
                        out=ajf[:, t:t + ln, :],
                        in0=aj_all[:, t:t + ln, :],
                        in1=fw3.to_broadcast((128, ln, N)),
                        op=mybir.AluOpType.mult,
                    )
                nc.vector.tensor_copy(
                    out=exv[:, i, :, 0:1],
                    in_=ajf[:, t, :].rearrange("p (g o) -> p g o", o=1),
                )
            eng = nc.sync if u % 2 == 0 else nc.scalar
            eng.dma_start(out=fat_v[u], in_=ex[:])
            # slot the weight/small loads onto the scalar ring behind its
            # first stores (they are needed only by the PE/epilogue phase)
            if u == 1:
                for tile_, src_ in wloads[:2]:
                    nc.scalar.dma_start(out=tile_[:], in_=src_[:])
            if u == 3:
                for tile_, src_ in wloads[2:]:
                    nc.scalar.dma_start(out=tile_[:], in_=src_[:])

        # fold father_weights into the father weight columns (per partition k,
        # the factor is fw[k % 64], precomputed on host as fw_col)
        wf_sb = singles.tile([128, KCH * A], f32)
        nc.vector.tensor_tensor(
            out=wf_sb[:], in0=wf_raw[:],
            in1=fwcol[:].to_broadcast((128, KCH * A)),
            op=mybir.AluOpType.mult,
        )

        # ---- PE transposes: 4 per PSUM tile, one DVE copy per group ----
        fsb = singles.tile([128, KCH * BL], f32)
        xt = singles.tile([128, DCH * BL], f32)
        srcs = [("f", c) for c in range(KCH)] + [("x", c) for c in range(DCH)]
        for g in range(0, len(srcs), 4):
            grp = srcs[g:g + 4]
            pt = psump.tile([128, 4, BL], f32, tag="pt", name="pt")
            for gi, (kind, c) in enumerate(grp):
                src = ab if kind == "f" else xb
                nc.tensor.transpose(
                    pt[:, gi, :], src[:, 128 * c:128 * (c + 1)], ident[:BL, :BL]
                )
            for gi, (kind, c) in enumerate(grp):
                dst = fsb if kind == "f" else xt
                nc.vector.tensor_copy(
                    out=dst[:, c * BL:(c + 1) * BL], in_=pt[:, gi, :]
                )

        lg = lgp.tile([BL, A], f32)
        for c in range(KCH):
            nc.tensor.matmul(
                lg[:],
                lhsT=fsb[:, BL * c:BL * (c + 1)],
                rhs=wf_sb[:, A * c:A * (c + 1)],
                start=(c == 0),
                stop=False,
            )
        for c in range(DCH):
            nc.tensor.matmul(
                lg[:],
                lhsT=xt[:, BL * c:BL * (c + 1)],
                rhs=wd_sb[:, A * c:A * (c + 1)],
                start=False,
                stop=(c == DCH - 1),
            )

        # ---- mask + log-softmax + argmax ----
        neg_sb = singles.tile([BL, A], f32)
        nc.vector.memset(neg_sb[:], NEG_INF)
        av_m = singles.tile([BL, A], mybir.dt.int8)
        nc.vector.tensor_scalar(
            out=av_m[:], in0=av_sb[:], scalar1=0.0, scalar2=None,
            op0=mybir.AluOpType.is_gt,
        )
        lg_sb = singles.tile([BL, A], f32)
        nc.vector.tensor_tensor(
            out=lg_sb[:], in0=lg[:], in1=b_sb[:], op=mybir.AluOpType.add
        )
        ml = singles.tile([BL, A], f32)
        nc.vector.tensor_copy(out=ml[:], in_=neg_sb[:])
        nc.vector.copy_predicated(out=ml[:], mask=av_m[:], data=lg_sb[:])

        m8 = singles.tile([BL, 8], f32)
        i8u = singles.tile([BL, 8], u32)
        nc.vector.max(m8[:], ml[:])
        nc.vector.max_index(i8u[:], m8[:], ml[:])

        sh = singles.tile([BL, A], f32)
        nc.vector.tensor_scalar(
            out=sh[:], in0=ml[:], scalar1=m8[:, 0:1], scalar2=None,
            op0=mybir.AluOpType.subtract,
        )
        et = singles.tile([BL, A], f32)
        ssum = singles.tile([BL, 1], f32)
        nc.scalar.activation(
            out=et[:], in_=sh[:], func=mybir.ActivationFunctionType.Exp,
            accum_out=ssum[:],
        )
        lns = singles.tile([BL, 1], f32)
        nc.scalar.activation(
            out=lns[:], in_=ssum[:], func=mybir.ActivationFunctionType.Ln
        )
        alp_sb = singles.tile([BL, 1], f32)
        nc.vector.tensor_scalar_mul(out=alp_sb[:], in0=lns[:], scalar1=-1.0)
        act_sb = singles.tile([BL, 1], i32)
        nc.vector.tensor_copy(out=act_sb[:], in_=i8u[:, 0:1])

        nc.sync.dma_start(out=act_d[:], in_=act_sb[:])
        nc.sync.dma_start(out=alp_d[:], in_=alp_sb[:])

    _legalize_multi_waits(nc, mybir)
    return nc


def _legalize_multi_waits(nc, mybir):
    """This toolchain's walrus accepts at most one embedded sync-wait per
    compute/DMA instruction (two for EventSemaphore).  Tile's sem assignment
    can emit more; spill the extras onto same-engine NoOp carriers inserted
    immediately before the instruction."""
    n = 0
    for fn in nc.m.functions:
        for blk in fn.blocks:
            insts = blk.instructions
            i = 0
            while i < len(insts):
                inst = insts[i]
                si = inst.sync_info
                cap = 2 if isinstance(inst, mybir.InstEventSemaphore) else 1
                if si is not None and len(si.on_wait) > cap:
                    waits = list(si.on_wait)
                    keep, extra = waits[-cap:], waits[:-cap]
                    inst.sync_info = mybir.SyncInfo(
                        on_wait=keep, on_update=list(si.on_update)
                    )
                    for w in extra:
                        n += 1
                        nop = mybir.InstNoOp(name=f"WSPLIT-{n}", ins=[], outs=[])
                        nop.engine = inst.engine
                        nop.sync_info = mybir.SyncInfo(on_wait=[w], on_update=[])
                        insts.insert(i, nop)
                        i += 1
                i += 1
    return n


def _get_nc():
    if "nc" not in _CACHE:
        _CACHE["nc"] = _build_nc()
    return _CACHE["nc"]


def _make_in_maps(x, adj, available_actions, father_weights, W, b):
    x = np.ascontiguousarray(x, dtype=np.float32)
    adj = np.ascontiguousarray(adj, dtype=np.int32)
    av = np.ascontiguousarray(available_actions, dtype=np.float32)
    fw = np.ascontiguousarray(father_weights, dtype=np.float32)
    W = np.ascontiguousarray(W, dtype=np.float32)
    b = np.ascontiguousarray(b, dtype=np.float32)

    # host-side relayout of the structurally-live weight columns (no math)
    wdt = np.ascontiguousarray(
        W[:, :D].T.reshape(DCH, 128, A).transpose(1, 0, 2).reshape(128, DCH * A)
    )
    wft = np.ascontiguousarray(
        W[:, D::A].T.reshape(KCH, 128, A).transpose(1, 0, 2).reshape(128, KCH * A)
    )
    fw_col = np.ascontiguousarray(np.concatenate([fw, fw])[:, None])
    fw_row = np.ascontiguousarray(fw[None, :])
    bias = np.ascontiguousarray(b[None, :])

    in_maps = []
    for i in range(M):
        sl = slice(BL * i, BL * (i + 1))
        in_maps.append({
            "adj": np.ascontiguousarray(adj[sl].reshape(R, N)),
            "x": np.ascontiguousarray(x[sl]),
            "avail": np.ascontiguousarray(av[sl]),
            "fw": fw_row,
            "fw_col": fw_col,
            "wft": wft,
            "wdt": wdt,
            "bias": bias,
        })
    return in_maps


def _assemble(results):
    actions = np.concatenate([r["actions"] for r in results], axis=0).astype(np.int32)
    alp = np.concatenate([r["alp"] for r in results], axis=0).astype(np.float32)
    father = np.concatenate(
        [r["father"].reshape(BL, NG * A) for r in results], axis=0
    ).astype(np.float32)
    return actions, alp, father


def run_on_device(inputs, trace=False, retries=2, **kw):
    """Compile+run the SPMD bass kernel; returns (outputs_tuple, BassKernelResults)."""
    import time

    from concourse.bass_utils import run_bass_kernel_spmd

    nc = _get_nc()
    in_maps = _make_in_maps(**inputs)
    last_err = None
    for attempt in range(retries + 1):
        try:
            res = run_bass_kernel_spmd(
                nc, in_maps, core_ids=list(range(M)), trace=trace, **kw
            )
            return _assemble(res.results), res
        except Exception as e:  # transient NRT/device errors recover on retry
            msg = str(e)
            if attempt < retries and (
                "UNRECOVERABLE" in msg or "UNAVAILABLE" in msg
                or "PassThrough" in msg
            ):
                last_err = e
                time.sleep(2.0)
                continue
            raise
    raise last_err


def kernel(**inputs):
    (actions, alp, father), _ = run_on_device(inputs, trace=False)
    return actions, alp, father


# revision 42
# speedup vs baseline: 1.0429x; 1.0101x over previous
"""Trainium2 Bass kernel for nn_ACTGraphLayer (gnn_message_passing).

Data-parallel over B=256 rollout threads: 32 rows per NeuronCore x 8 cores.

Per core:
  - father_flat shard [32, 65536] is viewed as [2048, 1024] rows; only every
    16th float (slot 0 of each action block) is nonzero: adj[b,j,p]*fw[p].
    Partition p owns output rows 16p..16p+15 so the adj load and all stores
    are contiguous per partition.  Zero-initialized [128, 1024] SBUF tiles
    get their stride-16 slot-0 lanes overwritten by strided vector copies,
    then stream out as 16 x 512 KB contiguous stores alternating across the
    two HWDGE rings (sync + scalar).
  - logits[b,a] = x@Wd.T + sum_{j,pp} adjf[b,j,pp]*Wf[(j,pp),a] + bias on
    the TensorEngine.  The PE path reuses ajf (fw already folded): for each
    row-group q, T_q = transpose(ajf[:,q,:]) puts parents pp on partitions
    with free index 4b+s, so the strided slice T_q[:, s::4] is exactly
    adjf[b, j=16s+q, :].T -- no second adj read and no extra convert.  64
    K=64 matmuls against j-grouped weight chunks (host relayout only, no
    host arithmetic) + 2 K=128 matmuls for the x part accumulate in PSUM.
    Softmax/argmax run on-device (max8/max_index + Exp/Ln).
"""

import os
import sys

import numpy as np

for _p in ("/opt/trn_rl_repo", "/root/.axon_site/_ro/trn_rl_repo"):
    if os.path.isdir(_p) and _p not in sys.path:
        sys.path.insert(0, _p)

B, N, A, D = 256, 64, 16, 256
M = 8                      # cores
BL = B // M                # 32 rows per core
NG = N * N                 # 4096 father groups per row
R = BL * N                 # 2048 output rows of 1024 floats per core
KCH = NG // 128            # 32 matmul K-chunks over the father axis
DCH = D // 128             # 2 matmul K-chunks over the x axis
NEG_INF = -1e10
TCH = R // 128             # 16 row-groups per partition
SUB = 1                    # row-groups per store (512 KB stores)
NST = TCH // SUB           # 16 stores
NEXP = 10                  # expansion tile ring depth

_CACHE = {}


def _build_nc():
    import concourse.bass as bass
    import concourse.mybir as mybir
    import concourse.tile as tile
    from concourse.masks import make_identity
    from contextlib import ExitStack

    f32 = mybir.dt.float32
    i32 = mybir.dt.int32
    u32 = mybir.dt.uint32

    nc = bass.Bass()
    adj_d = nc.declare_dram_parameter("adj", [R, N], i32, isOutput=False)
    x_d = nc.declare_dram_parameter("x", [BL, D], f32, isOutput=False)
    av_d = nc.declare_dram_parameter("avail", [BL, A], f32, isOutput=False)
    fw_d = nc.declare_dram_parameter("fw", [1, N], f32, isOutput=False)
    wfj_d = nc.declare_dram_parameter("wfj", [N, N * A], f32, False)
    wdt_d = nc.declare_dram_parameter("wdt", [128, DCH * A], f32, isOutput=False)
    b_d = nc.declare_dram_parameter("bias", [1, A], f32, isOutput=False)
    fat_d = nc.declare_dram_parameter("father", [R, N * A], f32, isOutput=True)
    act_d = nc.declare_dram_parameter("actions", [BL, 1], i32, isOutput=True)
    alp_d = nc.declare_dram_parameter("alp", [BL, 1], f32, isOutput=True)

    with ExitStack() as ctx:
        tc = ctx.enter_context(tile.TileContext(nc))
        singles = ctx.enter_context(tc.tile_pool(name="singles", bufs=1))
        expp = ctx.enter_context(tc.tile_pool(name="expp", bufs=1))
        psump = ctx.enter_context(tc.tile_pool(name="psump", bufs=6, space="PSUM"))
        lgp = ctx.enter_context(tc.tile_pool(name="lgp", bufs=1, space="PSUM"))

        # ---- early loads ----
        # sync ring: adj in expansion layout first (feeds the store stream).
        # Partition p holds output rows 16p..16p+15, so the load is fully
        # contiguous per partition (4 KB descriptors) and each store writes
        # 128 contiguous 4 KB runs.
        aj_all = singles.tile([128, TCH, N], i32)
        aj_src = adj_d[:].rearrange("(p t) n -> p t n", p=128)
        PIECES = [(0, 2), (2, 6), (8, 4), (12, 4)]  # (start group, n groups)
        for c, (s0, ln) in enumerate(PIECES):
            eng = nc.sync if c % 2 == 0 else nc.scalar
            eng.dma_start(
                out=aj_all[:, s0:s0 + ln, :], in_=aj_src[:, s0:s0 + ln, :]
            )
        # weight/small loads are emitted later (interleaved into the scalar
        # ring behind its first stores); declare tiles here
        wfj_sb = singles.tile([N, N, A], f32)
        xb = singles.tile([BL, D], f32)
        wd_sb = singles.tile([128, DCH * A], f32)
        av_sb = singles.tile([BL, A], f32)

        # POOL: broadcasts + identity + PE-path cast-load + half the memsets
        fwrow = singles.tile([128, N], f32)
        nc.gpsimd.dma_start(out=fwrow[:], in_=fw_d[:].to_broadcast((128, N)))
        ident = singles.tile([128, 128], f32)
        make_identity(nc, ident[:])
        exp_tiles = [
            expp.tile([128, SUB, N * A], f32, tag=f"exp{i}", name=f"exp{i}")
            for i in range(NEXP)
        ]
        # tiles are zeroed just-in-time inside the store loop (DVE/POOL split)

        b_sb = singles.tile([BL, A], f32)
        nc.gpsimd.dma_start(out=b_sb[:], in_=b_d[:].to_broadcast((BL, A)))

        # ---- expansion stream: ajf piece -> copies -> stores, interleaved ----
        ajf = singles.tile([128, TCH, N], f32)
        fw3 = fwrow[:].rearrange("p (o n) -> p o n", o=1)
        fat_v = fat_d[:].rearrange("(p u c) n -> u p c n", u=NST, c=SUB)
        piece_starts = {s0: ln for s0, ln in PIECES}
        wloads = [(wfj_sb, wfj_d), (xb, x_d), (wd_sb, wdt_d), (av_sb, av_d)]
        for u in range(NST):
            ex = exp_tiles[u % NEXP]
            if u < NEXP:
                (nc.vector if u % 2 == 0 else nc.gpsimd).memset(ex[:], 0.0)
            exv = ex[:].rearrange("p c (g a) -> p c g a", a=A)
            for i in range(SUB):
                t = SUB * u + i
                if t in piece_starts:
                    ln = piece_starts[t]
                    nc.vector.tensor_tensor(
                        out=ajf[:, t:t + ln, :],
                        in0=aj_all[:, t:t + ln, :],
                        in1=fw3.to_broadcast((128, ln, N)),
                        op=mybir.AluOpType.mult,
                    )
                nc.vector.tensor_copy(
                    out=exv[:, i, :, 0:1],
                    in_=ajf[:, t, :].rearrange("p (g o) -> p g o", o=1),
                )
            # ACT ring only takes early stores so it frees up for the
            # epilogue activations; SP takes the rest
            eng = nc.scalar if (u % 2 == 1 and u < 12) else nc.sync
            eng.dma_start(out=fat_v[u], in_=ex[:])
            # slot the weight/small loads onto the scalar ring behind its
            # first stores (they are needed only by the PE/epilogue phase)
            if u == 1:
                for tile_, src_ in wloads[:2]:
                    nc.scalar.dma_start(out=tile_[:], in_=src_[:])
            if u == 3:
                for tile_, src_ in wloads[2:]:
                    nc.scalar.dma_start(out=tile_[:], in_=src_[:])

        # ---- PE path: transpose ajf itself (fw already folded) ----
        # T_q[n, p] = ajf[p, q, n] = adjf[b=p//4, j=16*(p%4)+q, n], so the
        # strided free slice [:, s::4] of T_q is exactly adjf[b, j=16s+q, :].T
        # -- no second adj read and no separate f32 convert needed.
        tsb = singles.tile([N, TCH, 128], f32)
        for q in range(TCH):
            ptq = psump.tile([N, 128], f32, tag="pt", name="ptq")
            nc.tensor.transpose(ptq[:], ajf[:, q, :], ident[:, :])
            nc.vector.tensor_copy(out=tsb[:, q, :], in_=ptq[:])
        xt = singles.tile([128, DCH * BL], f32)
        for c in range(DCH):
            ptx = psump.tile([128, BL], f32, tag="pt", name="ptx")
            nc.tensor.transpose(
                ptx[:], xb[:, 128 * c:128 * (c + 1)], ident[:BL, :BL]
            )
            nc.vector.tensor_copy(
                out=xt[:, c * BL:(c + 1) * BL], in_=ptx[:]
            )

        lg = lgp.tile([BL, A], f32)
        for j in range(N):
            s, q = j // TCH, j % TCH
            lhs = tsb[:, q, :].rearrange("p (b s) -> p s b", s=4)[:, s, :]
            nc.tensor.matmul(
                lg[:], lhsT=lhs, rhs=wfj_sb[:, j, :],
                start=(j == 0), stop=False,
            )
        for c in range(DCH):
            nc.tensor.matmul(
                lg[:],
                lhsT=xt[:, BL * c:BL * (c + 1)],
                rhs=wd_sb[:, A * c:A * (c + 1)],
                start=False,
                stop=(c == DCH - 1),
            )

        # ---- mask + log-softmax + argmax ----
        neg_sb = singles.tile([BL, A], f32)
        nc.vector.memset(neg_sb[:], NEG_INF)
        av_m = singles.tile([BL, A], mybir.dt.int8)
        nc.vector.tensor_scalar(
            out=av_m[:], in0=av_sb[:], scalar1=0.0, scalar2=None,
            op0=mybir.AluOpType.is_gt,
        )
        lg_sb = singles.tile([BL, A], f32)
        nc.vector.tensor_tensor(
            out=lg_sb[:], in0=lg[:], in1=b_sb[:], op=mybir.AluOpType.add
        )
        ml = singles.tile([BL, A], f32)
        nc.vector.tensor_copy(out=ml[:], in_=neg_sb[:])
        nc.vector.copy_predicated(out=ml[:], mask=av_m[:], data=lg_sb[:])

        m8 = singles.tile([BL, 8], f32)
        i8u = singles.tile([BL, 8], u32)
        nc.vector.max(m8[:], ml[:])
        nc.vector.max_index(i8u[:], m8[:], ml[:])

        sh = singles.tile([BL, A], f32)
        nc.vector.tensor_scalar(
            out=sh[:], in0=ml[:], scalar1=m8[:, 0:1], scalar2=None,
            op0=mybir.AluOpType.subtract,
        )
        et = singles.tile([BL, A], f32)
        ssum = singles.tile([BL, 1], f32)
        nc.scalar.activation(
            out=et[:], in_=sh[:], func=mybir.ActivationFunctionType.Exp,
            accum_out=ssum[:],
        )
        lns = singles.tile([BL, 1], f32)
        nc.scalar.activation(
            out=lns[:], in_=ssum[:], func=mybir.ActivationFunctionType.Ln
        )
        alp_sb = singles.tile([BL, 1], f32)
        nc.vector.tensor_scalar_mul(out=alp_sb[:], in0=lns[:], scalar1=-1.0)
        act_sb = singles.tile([BL, 1], i32)
        nc.vector.tensor_copy(out=act_sb[:], in_=i8u[:, 0:1])

        nc.scalar.dma_start(out=act_d[:], in_=act_sb[:])
        nc.scalar.dma_start(out=alp_d[:], in_=alp_sb[:])

    _legalize_multi_waits(nc, mybir)
    return nc


def _legalize_multi_waits(nc, mybir):
    """This toolchain's walrus accepts at most one embedded sync-wait per
    compute/DMA instruction (two for EventSemaphore).  Tile's sem assignment
    can emit more; spill the extras onto same-engine NoOp carriers inserted
    immediately before the instruction."""
    n = 0
    for fn in nc.m.functions:
        for blk in fn.blocks:
            insts = blk.instructions
            i = 0
            while i < len(insts):
                inst = insts[i]
                si = inst.sync_info
                cap = 2 if isinstance(inst, mybir.InstEventSemaphore) else 1
                if si is not None and len(si.on_wait) > cap:
                    waits = list(si.on_wait)
                    keep, extra = waits[-cap:], waits[:-cap]
                    inst.sync_info = mybir.SyncInfo(
                        on_wait=keep, on_update=list(si.on_update)
                    )
                    for w in extra:
                        n += 1
                        nop = mybir.InstNoOp(name=f"WSPLIT-{n}", ins=[], outs=[])
                        nop.engine = inst.engine
                        nop.sync_info = mybir.SyncInfo(on_wait=[w], on_update=[])
                        insts.insert(i, nop)
                        i += 1
                i += 1
    return n


def _get_nc():
    if "nc" not in _CACHE:
        _CACHE["nc"] = _build_nc()
    return _CACHE["nc"]


def _make_in_maps(x, adj, available_actions, father_weights, W, b):
    x = np.ascontiguousarray(x, dtype=np.float32)
    adj = np.ascontiguousarray(adj, dtype=np.int32)
    av = np.ascontiguousarray(available_actions, dtype=np.float32)
    fw = np.ascontiguousarray(father_weights, dtype=np.float32)
    W = np.ascontiguousarray(W, dtype=np.float32)
    b = np.ascontiguousarray(b, dtype=np.float32)

    # host-side relayout of the structurally-live weight columns (no math)
    wdt = np.ascontiguousarray(
        W[:, :D].T.reshape(DCH, 128, A).transpose(1, 0, 2).reshape(128, DCH * A)
    )
    # wfj[pp, j, a] = W[a, 256 + 16*(j*64+pp)]  (pure relayout, no math)
    wfj = np.ascontiguousarray(
        W[:, D::A].T.reshape(N, N, A).transpose(1, 0, 2).reshape(N, N * A)
    )
    fw_row = np.ascontiguousarray(fw[None, :])
    bias = np.ascontiguousarray(b[None, :])

    in_maps = []
    for i in range(M):
        sl = slice(BL * i, BL * (i + 1))
        in_maps.append({
            "adj": np.ascontiguousarray(adj[sl].reshape(R, N)),
            "x": np.ascontiguousarray(x[sl]),
            "avail": np.ascontiguousarray(av[sl]),
            "fw": fw_row,
            "wfj": wfj,
            "wdt": wdt,
            "bias": bias,
        })
    return in_maps


def _assemble(results):
    actions = np.concatenate([r["actions"] for r in results], axis=0).astype(np.int32)
    alp = np.concatenate([r["alp"] for r in results], axis=0).astype(np.float32)
    father = np.concatenate(
        [r["father"].reshape(BL, NG * A) for r in results], axis=0
    ).astype(np.float32)
    return actions, alp, father


def run_on_device(inputs, trace=False, retries=2, **kw):
    """Compile+run the SPMD bass kernel; returns (outputs_tuple, BassKernelResults)."""
    import time

    from concourse.bass_utils import run_bass_kernel_spmd

    nc = _get_nc()
    in_maps = _make_in_maps(**inputs)
    last_err = None
    for attempt in range(retries + 1):
        try:
            res = run_bass_kernel_spmd(
                nc, in_maps, core_ids=list(range(M)), trace=trace, **kw
            )
            return _assemble(res.results), res
        except Exception as e:  # transient NRT/device errors recover on retry
            msg = str(e)
            if attempt < retries and (
                "UNRECOVERABLE" in msg or "UNAVAILABLE" in msg
                or "PassThrough" in msg
            ):
                last_err = e
                time.sleep(2.0)
                continue
            raise
    raise last_err


def kernel(**inputs):
    (actions, alp, father), _ = run_on_device(inputs, trace=False)
    return actions, alp, father


# revision 48
# speedup vs baseline: 1.0741x; 1.0299x over previous
"""Trainium2 Bass kernel for nn_ACTGraphLayer (gnn_message_passing).

Data-parallel over B=256 rollout threads: 32 rows per NeuronCore x 8 cores.

Per core:
  - father_flat shard [32, 65536] is viewed as [2048, 1024] rows; only every
    16th float (slot 0 of each action block) is nonzero: adj[b,j,p]*fw[p].
    Partition p owns output rows 16p..16p+15 so the adj load and all stores
    are contiguous per partition.  Zero-initialized [128, 1024] SBUF tiles
    get their stride-16 slot-0 lanes overwritten by strided vector copies,
    then stream out as 16 x 512 KB contiguous stores alternating across the
    two HWDGE rings (sync + scalar).
  - logits[b,a] = x@Wd.T + sum_f adj[b,f]*fw[f%64]*Wf[f,a] + bias, computed
    on the TensorEngine (PE transposes of adj/x feed 34 accumulating
    matmuls).  Only the structurally-live columns of W are shipped to the
    device (pure host-side slicing/relayout of the weight tensor - no host
    arithmetic).  Softmax/argmax run on-device (max8/max_index + Exp/Ln).
"""

import os
import sys

import numpy as np

for _p in ("/opt/trn_rl_repo", "/root/.axon_site/_ro/trn_rl_repo"):
    if os.path.isdir(_p) and _p not in sys.path:
        sys.path.insert(0, _p)

B, N, A, D = 256, 64, 16, 256
M = 8                      # cores
BL = B // M                # 32 rows per core
NG = N * N                 # 4096 father groups per row
R = BL * N                 # 2048 output rows of 1024 floats per core
KCH = NG // 128            # 32 matmul K-chunks over the father axis
DCH = D // 128             # 2 matmul K-chunks over the x axis
NEG_INF = -1e10
TCH = R // 128             # 16 row-groups per partition
SUB = 1                    # row-groups per store (512 KB stores)
NST = TCH // SUB           # 16 stores
NEXP = 10                  # expansion tile ring depth

_CACHE = {}


def _build_nc():
    import concourse.bass as bass
    import concourse.mybir as mybir
    import concourse.tile as tile
    from concourse.masks import make_identity
    from contextlib import ExitStack

    f32 = mybir.dt.float32
    i32 = mybir.dt.int32
    u32 = mybir.dt.uint32

    nc = bass.Bass()
    adj_d = nc.declare_dram_parameter("adj", [R, N], i32, isOutput=False)
    x_d = nc.declare_dram_parameter("x", [BL, D], f32, isOutput=False)
    av_d = nc.declare_dram_parameter("avail", [BL, A], f32, isOutput=False)
    fw_d = nc.declare_dram_parameter("fw", [1, N], f32, isOutput=False)
    wfj_d = nc.declare_dram_parameter("wfj", [N, N * A], f32, False)
    wdt_d = nc.declare_dram_parameter("wdt", [128, DCH * A], f32, isOutput=False)
    b_d = nc.declare_dram_parameter("bias", [1, A], f32, isOutput=False)
    fat_d = nc.declare_dram_parameter("father", [R, N * A], f32, isOutput=True)
    act_d = nc.declare_dram_parameter("actions", [BL, 1], i32, isOutput=True)
    alp_d = nc.declare_dram_parameter("alp", [BL, 1], f32, isOutput=True)

    with ExitStack() as ctx:
        tc = ctx.enter_context(tile.TileContext(nc))
        singles = ctx.enter_context(tc.tile_pool(name="singles", bufs=1))
        expp = ctx.enter_context(tc.tile_pool(name="expp", bufs=1))
        psump = ctx.enter_context(tc.tile_pool(name="psump", bufs=6, space="PSUM"))
        lgp = ctx.enter_context(tc.tile_pool(name="lgp", bufs=1, space="PSUM"))

        # ---- early loads ----
        # sync ring: adj in expansion layout first (feeds the store stream).
        # Partition p holds output rows 16p..16p+15, so the load is fully
        # contiguous per partition (4 KB descriptors) and each store writes
        # 128 contiguous 4 KB runs.
        aj_all = singles.tile([128, TCH, N], i32)
        aj_src = adj_d[:].rearrange("(p t) n -> p t n", p=128)
        PIECES = [(0, 2), (2, 6), (8, 4), (12, 4)]  # (start group, n groups)
        for c, (s0, ln) in enumerate(PIECES):
            eng = nc.sync if c % 2 == 0 else nc.scalar
            eng.dma_start(
                out=aj_all[:, s0:s0 + ln, :], in_=aj_src[:, s0:s0 + ln, :]
            )
        # weight/small loads are emitted later (interleaved into the scalar
        # ring behind its first stores); declare tiles here
        wfj_sb = singles.tile([N, N, A], f32)
        xb = singles.tile([BL, D], f32)
        wd_sb = singles.tile([128, DCH * A], f32)
        av_sb = singles.tile([BL, A], f32)

        # POOL: broadcasts + identity + PE-path cast-load + half the memsets
        fwrow = singles.tile([128, N], f32)
        nc.gpsimd.dma_start(out=fwrow[:], in_=fw_d[:].to_broadcast((128, N)))
        ident = singles.tile([128, 128], f32)
        make_identity(nc, ident[:])
        exp_tiles = [
            expp.tile([128, SUB, N * A], f32, tag=f"exp{i}", name=f"exp{i}")
            for i in range(NEXP)
        ]
        # tiles are zeroed just-in-time inside the store loop (DVE/POOL split)

        b_sb = singles.tile([BL, A], f32)
        nc.gpsimd.dma_start(out=b_sb[:], in_=b_d[:].to_broadcast((BL, A)))

        # ---- expansion stream: ajf piece -> copies -> stores, interleaved ----
        ajf = singles.tile([128, TCH, N], f32)
        fw3 = fwrow[:].rearrange("p (o n) -> p o n", o=1)
        fat_v = fat_d[:].rearrange("(p u c) n -> u p c n", u=NST, c=SUB)
        piece_starts = {s0: ln for s0, ln in PIECES}
        wloads = [(wfj_sb, wfj_d), (xb, x_d), (wd_sb, wdt_d), (av_sb, av_d)]
        for u in range(NST):
            ex = exp_tiles[u % NEXP]
            if u < NEXP:
                (nc.vector if u % 2 == 0 else nc.gpsimd).memset(ex[:], 0.0)
            exv = ex[:].rearrange("p c (g a) -> p c g a", a=A)
            for i in range(SUB):
                t = SUB * u + i
                if t in piece_starts:
                    ln = piece_starts[t]
                    nc.vector.tensor_tensor(
                        out=ajf[:, t:t + ln, :],
                        in0=aj_all[:, t:t + ln, :],
                        in1=fw3.to_broadcast((128, ln, N)),
                        op=mybir.AluOpType.mult,
                    )
                nc.vector.tensor_copy(
                    out=exv[:, i, :, 0:1],
                    in_=ajf[:, t, :].rearrange("p (g o) -> p g o", o=1),
                )
            # ACT ring only takes early stores so it frees up for the
            # epilogue activations; SP takes the rest
            eng = nc.scalar if (u % 2 == 1 and u < 12) else nc.sync
            eng.dma_start(out=fat_v[u], in_=ex[:])
            # slot the weight/small loads onto the scalar ring behind its
            # first stores (they are needed only by the PE/epilogue phase)
            if u == 1:
                for tile_, src_ in wloads[:2]:
                    nc.scalar.dma_start(out=tile_[:], in_=src_[:])
            if u == 3:
                for tile_, src_ in wloads[2:]:
                    nc.scalar.dma_start(out=tile_[:], in_=src_[:])

        # ---- PE path: transpose ajf itself (fw already folded) ----
        # T_q[n, p] = ajf[p, q, n] = adjf[b=p//4, j=16*(p%4)+q, n], so the
        # strided free slice [:, s::4] of T_q is exactly adjf[b, j=16s+q, :].T
        # -- no second adj read and no separate f32 convert needed.
        tsb = singles.tile([N, TCH, 128], f32)
        for q in range(TCH):
            ptq = psump.tile([N, 128], f32, tag="pt", name="ptq")
            nc.tensor.transpose(ptq[:], ajf[:, q, :], ident[:, :])
            nc.vector.tensor_copy(out=tsb[:, q, :], in_=ptq[:])
        xt = singles.tile([128, DCH * BL], f32)
        for c in range(DCH):
            ptx = psump.tile([128, BL], f32, tag="pt", name="ptx")
            nc.tensor.transpose(
                ptx[:], xb[:, 128 * c:128 * (c + 1)], ident[:BL, :BL]
            )
            nc.vector.tensor_copy(
                out=xt[:, c * BL:(c + 1) * BL], in_=ptx[:]
            )

        lg = lgp.tile([BL, A], f32)
        for j in range(N):
            s, q = j // TCH, j % TCH
            lhs = tsb[:, q, :].rearrange("p (b s) -> p s b", s=4)[:, s, :]
            nc.tensor.matmul(
                lg[:], lhsT=lhs, rhs=wfj_sb[:, j, :],
                start=(j == 0), stop=False,
            )
        for c in range(DCH):
            nc.tensor.matmul(
                lg[:],
                lhsT=xt[:, BL * c:BL * (c + 1)],
                rhs=wd_sb[:, A * c:A * (c + 1)],
                start=False,
                stop=(c == DCH - 1),
            )

        # ---- mask + log-softmax + argmax ----
        neg_sb = singles.tile([BL, A], f32)
        nc.vector.memset(neg_sb[:], NEG_INF)
        av_m = singles.tile([BL, A], mybir.dt.int8)
        nc.vector.tensor_scalar(
            out=av_m[:], in0=av_sb[:], scalar1=0.0, scalar2=None,
            op0=mybir.AluOpType.is_gt,
        )
        lg_sb = singles.tile([BL, A], f32)
        nc.vector.tensor_tensor(
            out=lg_sb[:], in0=lg[:], in1=b_sb[:], op=mybir.AluOpType.add
        )
        ml = singles.tile([BL, A], f32)
        nc.vector.tensor_copy(out=ml[:], in_=neg_sb[:])
        nc.vector.copy_predicated(out=ml[:], mask=av_m[:], data=lg_sb[:])

        m8 = singles.tile([BL, 8], f32)
        i8u = singles.tile([BL, 8], u32)
        nc.vector.max(m8[:], ml[:])
        nc.vector.max_index(i8u[:], m8[:], ml[:])

        sh = singles.tile([BL, A], f32)
        nc.vector.tensor_scalar(
            out=sh[:], in0=ml[:], scalar1=m8[:, 0:1], scalar2=None,
            op0=mybir.AluOpType.subtract,
        )
        et = singles.tile([BL, A], f32)
        ssum = singles.tile([BL, 1], f32)
        nc.scalar.activation(
            out=et[:], in_=sh[:], func=mybir.ActivationFunctionType.Exp,
            accum_out=ssum[:],
        )
        lns = singles.tile([BL, 1], f32)
        nc.scalar.activation(
            out=lns[:], in_=ssum[:], func=mybir.ActivationFunctionType.Ln
        )
        alp_sb = singles.tile([BL, 1], f32)
        nc.vector.tensor_scalar_mul(out=alp_sb[:], in0=lns[:], scalar1=-1.0)
        act_sb = singles.tile([BL, 1], i32)
        nc.vector.tensor_copy(out=act_sb[:], in_=i8u[:, 0:1])

        nc.scalar.dma_start(out=act_d[:], in_=act_sb[:])
        nc.scalar.dma_start(out=alp_d[:], in_=alp_sb[:])

    _legalize_multi_waits(nc, mybir)
    return nc


def _legalize_multi_waits(nc, mybir):
    """This toolchain's walrus accepts at most one embedded sync-wait per
    compute/DMA instruction (two for EventSemaphore).  Tile's sem assignment
    can emit more; spill the extras onto same-engine NoOp carriers inserted
    immediately before the instruction."""
    n = 0
    for fn in nc.m.functions:
        for blk in fn.blocks:
            insts = blk.instructions
            i = 0
            while i < len(insts):
                inst = insts[i]
                si = inst.sync_info
                cap = 2 if isinstance(inst, mybir.InstEventSemaphore) else 1
                if si is not None and len(si.on_wait) > cap:
                    waits = list(si.on_wait)
                    keep, extra = waits[-cap:], waits[:-cap]
                    inst.sync_info = mybir.SyncInfo(
                        on_wait=keep, on_update=list(si.on_update)
                    )
                    for w in extra:
                        n += 1
                        nop = mybir.InstNoOp(name=f"WSPLIT-{n}", ins=[], outs=[])
                        nop.engine = inst.engine
                        nop.sync_info = mybir.SyncInfo(on_wait=[w], on_update=[])
                        insts.insert(i, nop)
                        i += 1
                i += 1
    return n


def _get_nc():
    if "nc" not in _CACHE:
        _CACHE["nc"] = _build_nc()
    return _CACHE["nc"]


def _make_in_maps(x, adj, available_actions, father_weights, W, b):
    x = np.ascontiguousarray(x, dtype=np.float32)
    adj = np.ascontiguousarray(adj, dtype=np.int32)
    av = np.ascontiguousarray(available_actions, dtype=np.float32)
    fw = np.ascontiguousarray(father_weights, dtype=np.float32)
    W = np.ascontiguousarray(W, dtype=np.float32)
    b = np.ascontiguousarray(b, dtype=np.float32)

    # host-side relayout of the structurally-live weight columns (no math)
    wdt = np.ascontiguousarray(
        W[:, :D].T.reshape(DCH, 128, A).transpose(1, 0, 2).reshape(128, DCH * A)
    )
    # wfj[pp, j, a] = W[a, 256 + 16*(j*64+pp)]  (pure relayout, no math)
    wfj = np.ascontiguousarray(
        W[:, D::A].T.reshape(N, N, A).transpose(1, 0, 2).reshape(N, N * A)
    )
    fw_row = np.ascontiguousarray(fw[None, :])
    bias = np.ascontiguousarray(b[None, :])

    in_maps = []
    for i in range(M):
        sl = slice(BL * i, BL * (i + 1))
        in_maps.append({
            "adj": np.ascontiguousarray(adj[sl].reshape(R, N)),
            "x": np.ascontiguousarray(x[sl]),
            "avail": np.ascontiguousarray(av[sl]),
            "fw": fw_row,
            "wfj": wfj,
            "wdt": wdt,
            "bias": bias,
        })
    return in_maps


def _assemble(results):
    actions = np.concatenate([r["actions"] for r in results], axis=0).astype(np.int32)
    alp = np.concatenate([r["alp"] for r in results], axis=0).astype(np.float32)
    father = np.concatenate(
        [r["father"].reshape(BL, NG * A) for r in results], axis=0
    ).astype(np.float32)
    return actions, alp, father


def run_on_device(inputs, trace=False, retries=2, **kw):
    """Compile+run the SPMD bass kernel; returns (outputs_tuple, BassKernelResults)."""
    import time

    from concourse.bass_utils import run_bass_kernel_spmd

    nc = _get_nc()
    in_maps = _make_in_maps(**inputs)
    last_err = None
    for attempt in range(retries + 1):
        try:
            res = run_bass_kernel_spmd(
                nc, in_maps, core_ids=list(range(M)), trace=trace, **kw
            )
            return _assemble(res.results), res
        except Exception as e:  # transient NRT/device errors recover on retry
            msg = str(e)
            if attempt < retries and (
                "UNRECOVERABLE" in msg or "UNAVAILABLE" in msg
                or "PassThrough" in msg
            ):
                last_err = e
                time.sleep(2.0)
                continue
            raise
    raise last_err


def kernel(**inputs):
    (actions, alp, father), _ = run_on_device(inputs, trace=False)
    return actions, alp, father


# revision 49
# speedup vs baseline: 1.0927x; 1.0174x over previous
"""Trainium2 Bass kernel for nn_ACTGraphLayer (gnn_message_passing).

Data-parallel over B=256 rollout threads: 32 rows per NeuronCore x 8 cores.

Per core:
  - father_flat shard [32, 65536] is viewed as [2048, 1024] rows; only every
    16th float (slot 0 of each action block) is nonzero: adj[b,j,p]*fw[p].
    Partition p owns output rows 16p..16p+15 so the adj load and all stores
    are contiguous per partition.  Zero-initialized [128, 1024] SBUF tiles
    get their stride-16 slot-0 lanes overwritten by strided vector copies,
    then stream out as 16 x 512 KB contiguous stores alternating across the
    two HWDGE rings (sync + scalar).
  - logits[b,a] = x@Wd.T + sum_{j,pp} adjf[b,j,pp]*Wf[(j,pp),a] + bias on
    the TensorEngine.  The PE path reuses ajf (fw already folded): for each
    row-group q, T_q = transpose(ajf[:,q,:]) puts parents pp on partitions
    with free index 4b+s, so the strided slice T_q[:, s::4] is exactly
    adjf[b, j=16s+q, :].T -- no second adj read and no extra convert.  64
    K=64 matmuls against j-grouped weight chunks (host relayout only, no
    host arithmetic) + 2 K=128 matmuls for the x part accumulate in PSUM.
    Softmax/argmax run on-device (max8/max_index + Exp/Ln).
"""

import os
import sys

import numpy as np

for _p in ("/opt/trn_rl_repo", "/root/.axon_site/_ro/trn_rl_repo"):
    if os.path.isdir(_p) and _p not in sys.path:
        sys.path.insert(0, _p)

B, N, A, D = 256, 64, 16, 256
M = 8                      # cores
BL = B // M                # 32 rows per core
NG = N * N                 # 4096 father groups per row
R = BL * N                 # 2048 output rows of 1024 floats per core
KCH = NG // 128            # 32 matmul K-chunks over the father axis
DCH = D // 128             # 2 matmul K-chunks over the x axis
NEG_INF = -1e10
TCH = R // 128             # 16 row-groups per partition
SUB = 1                    # row-groups per store (512 KB stores)
NST = TCH // SUB           # 16 stores
NEXP = 10                  # expansion tile ring depth

_CACHE = {}


def _build_nc():
    import concourse.bass as bass
    import concourse.mybir as mybir
    import concourse.tile as tile
    from concourse.masks import make_identity
    from contextlib import ExitStack

    f32 = mybir.dt.float32
    i32 = mybir.dt.int32
    u32 = mybir.dt.uint32

    nc = bass.Bass()
    adj_d = nc.declare_dram_parameter("adj", [R, N], i32, isOutput=False)
    x_d = nc.declare_dram_parameter("x", [BL, D], f32, isOutput=False)
    av_d = nc.declare_dram_parameter("avail", [BL, A], f32, isOutput=False)
    fw_d = nc.declare_dram_parameter("fw", [1, N], f32, isOutput=False)
    wfj_d = nc.declare_dram_parameter("wfj", [N, N * A], f32, False)
    wdt_d = nc.declare_dram_parameter("wdt", [128, DCH * A], f32, isOutput=False)
    b_d = nc.declare_dram_parameter("bias", [1, A], f32, isOutput=False)
    fat_d = nc.declare_dram_parameter("father", [R, N * A], f32, isOutput=True)
    act_d = nc.declare_dram_parameter("actions", [BL, 1], i32, isOutput=True)
    alp_d = nc.declare_dram_parameter("alp", [BL, 1], f32, isOutput=True)

    with ExitStack() as ctx:
        tc = ctx.enter_context(tile.TileContext(nc))
        singles = ctx.enter_context(tc.tile_pool(name="singles", bufs=1))
        expp = ctx.enter_context(tc.tile_pool(name="expp", bufs=1))
        psump = ctx.enter_context(tc.tile_pool(name="psump", bufs=6, space="PSUM"))
        lgp = ctx.enter_context(tc.tile_pool(name="lgp", bufs=1, space="PSUM"))

        # ---- early loads ----
        # sync ring: adj in expansion layout first (feeds the store stream).
        # Partition p holds output rows 16p..16p+15, so the load is fully
        # contiguous per partition (4 KB descriptors) and each store writes
        # 128 contiguous 4 KB runs.
        aj_all = singles.tile([128, TCH, N], i32)
        aj_src = adj_d[:].rearrange("(p t) n -> p t n", p=128)
        PIECES = [(0, 2), (2, 6), (8, 4), (12, 4)]  # (start group, n groups)
        for c, (s0, ln) in enumerate(PIECES):
            eng = nc.sync if c % 2 == 0 else nc.scalar
            eng.dma_start(
                out=aj_all[:, s0:s0 + ln, :], in_=aj_src[:, s0:s0 + ln, :]
            )
        # weight/small loads are emitted later (interleaved into the scalar
        # ring behind its first stores); declare tiles here
        wfj_sb = singles.tile([N, N, A], f32)
        xb = singles.tile([BL, D], f32)
        wd_sb = singles.tile([128, DCH * A], f32)
        av_sb = singles.tile([BL, A], f32)

        # POOL: broadcasts + identity + PE-path cast-load + half the memsets
        fwrow = singles.tile([128, N], f32)
        nc.gpsimd.dma_start(out=fwrow[:], in_=fw_d[:].to_broadcast((128, N)))
        ident = singles.tile([128, 128], f32)
        make_identity(nc, ident[:])
        exp_tiles = [
            expp.tile([128, SUB, N * A], f32, tag=f"exp{i}", name=f"exp{i}")
            for i in range(NEXP)
        ]
        # tiles are zeroed just-in-time inside the store loop (DVE/POOL split)

        b_sb = singles.tile([BL, A], f32)
        nc.gpsimd.dma_start(out=b_sb[:], in_=b_d[:].to_broadcast((BL, A)))

        # ---- expansion stream: ajf piece -> copies -> stores, interleaved ----
        ajf = singles.tile([128, TCH, N], f32)
        fw3 = fwrow[:].rearrange("p (o n) -> p o n", o=1)
        fat_v = fat_d[:].rearrange("(p u c) n -> u p c n", u=NST, c=SUB)
        piece_starts = {s0: ln for s0, ln in PIECES}
        wloads = [(wfj_sb, wfj_d), (xb, x_d), (wd_sb, wdt_d), (av_sb, av_d)]
        for u in range(NST):
            ex = exp_tiles[u % NEXP]
            if u < NEXP:
                (nc.vector if u % 2 == 0 else nc.gpsimd).memset(ex[:], 0.0)
            exv = ex[:].rearrange("p c (g a) -> p c g a", a=A)
            for i in range(SUB):
                t = SUB * u + i
                if t in piece_starts:
                    ln = piece_starts[t]
                    nc.vector.tensor_tensor(
                        out=ajf[:, t:t + ln, :],
                        in0=aj_all[:, t:t + ln, :],
                        in1=fw3.to_broadcast((128, ln, N)),
                        op=mybir.AluOpType.mult,
                    )
                nc.vector.tensor_copy(
                    out=exv[:, i, :, 0:1],
                    in_=ajf[:, t, :].rearrange("p (g o) -> p g o", o=1),
                )
            # ACT ring only takes early stores so it frees up for the
            # epilogue activations; SP takes the rest
            eng = nc.scalar if (u % 2 == 1 and u < 12) else nc.sync
            eng.dma_start(out=fat_v[u], in_=ex[:])
            # slot the weight/small loads onto the scalar ring behind its
            # first stores (they are needed only by the PE/epilogue phase)
            if u == 1:
                for tile_, src_ in wloads[:2]:
                    nc.scalar.dma_start(out=tile_[:], in_=src_[:])
            if u == 3:
                for tile_, src_ in wloads[2:]:
                    nc.scalar.dma_start(out=tile_[:], in_=src_[:])

        # ---- PE path: transpose ajf itself (fw already folded) ----
        # T_q[n, p] = ajf[p, q, n] = adjf[b=p//4, j=16*(p%4)+q, n], so the
        # strided free slice [:, s::4] of T_q is exactly adjf[b, j=16s+q, :].T
        # -- no second adj read and no separate f32 convert needed.
        tsb = singles.tile([N, TCH, 128], f32)
        for q in range(TCH):
            ptq = psump.tile([N, 128], f32, tag="pt", name="ptq")
            nc.tensor.transpose(ptq[:], ajf[:, q, :], ident[:, :])
            nc.vector.tensor_copy(out=tsb[:, q, :], in_=ptq[:])
        xt = singles.tile([128, DCH * BL], f32)
        for c in range(DCH):
            ptx = psump.tile([128, BL], f32, tag="pt", name="ptx")
            nc.tensor.transpose(
                ptx[:], xb[:, 128 * c:128 * (c + 1)], ident[:BL, :BL]
            )
            nc.vector.tensor_copy(
                out=xt[:, c * BL:(c + 1) * BL], in_=ptx[:]
            )

        lg = lgp.tile([BL, A], f32)
        for j in range(N):
            s, q = j // TCH, j % TCH
            lhs = tsb[:, q, :].rearrange("p (b s) -> p s b", s=4)[:, s, :]
            nc.tensor.matmul(
                lg[:], lhsT=lhs, rhs=wfj_sb[:, j, :],
                start=(j == 0), stop=False,
            )
        for c in range(DCH):
            nc.tensor.matmul(
                lg[:],
                lhsT=xt[:, BL * c:BL * (c + 1)],
                rhs=wd_sb[:, A * c:A * (c + 1)],
                start=False,
                stop=(c == DCH - 1),
            )

        # ---- mask + log-softmax + argmax ----
        neg_sb = singles.tile([BL, A], f32)
        nc.vector.memset(neg_sb[:], NEG_INF)
        av_m = singles.tile([BL, A], mybir.dt.int8)
        nc.vector.tensor_scalar(
            out=av_m[:], in0=av_sb[:], scalar1=0.0, scalar2=None,
            op0=mybir.AluOpType.is_gt,
        )
        lg_sb = singles.tile([BL, A], f32)
        nc.vector.tensor_tensor(
            out=lg_sb[:], in0=lg[:], in1=b_sb[:], op=mybir.AluOpType.add
        )
        ml = singles.tile([BL, A], f32)
        nc.vector.tensor_copy(out=ml[:], in_=neg_sb[:])
        nc.vector.copy_predicated(out=ml[:], mask=av_m[:], data=lg_sb[:])

        m8 = singles.tile([BL, 8], f32)
        i8u = singles.tile([BL, 8], u32)
        nc.vector.max(m8[:], ml[:])
        nc.vector.max_index(i8u[:], m8[:], ml[:])

        sh = singles.tile([BL, A], f32)
        nc.vector.tensor_scalar(
            out=sh[:], in0=ml[:], scalar1=m8[:, 0:1], scalar2=None,
            op0=mybir.AluOpType.subtract,
        )
        et = singles.tile([BL, A], f32)
        ssum = singles.tile([BL, 1], f32)
        nc.scalar.activation(
            out=et[:], in_=sh[:], func=mybir.ActivationFunctionType.Exp,
            accum_out=ssum[:],
        )
        lns = singles.tile([BL, 1], f32)
        nc.scalar.activation(
            out=lns[:], in_=ssum[:], func=mybir.ActivationFunctionType.Ln
        )
        alp_sb = singles.tile([BL, 1], f32)
        nc.vector.tensor_scalar_mul(out=alp_sb[:], in0=lns[:], scalar1=-1.0)
        act_sb = singles.tile([BL, 1], i32)
        nc.vector.tensor_copy(out=act_sb[:], in_=i8u[:, 0:1])

        nc.scalar.dma_start(out=act_d[:], in_=act_sb[:])
        nc.scalar.dma_start(out=alp_d[:], in_=alp_sb[:])

    _legalize_multi_waits(nc, mybir)
    return nc


def _legalize_multi_waits(nc, mybir):
    """This toolchain's walrus accepts at most one embedded sync-wait per
    compute/DMA instruction (two for EventSemaphore).  Tile's sem assignment
    can emit more; spill the extras onto same-engine NoOp carriers inserted
    immediately before the instruction."""
    n = 0
    for fn in nc.m.functions:
        for blk in fn.blocks:
            insts = blk.instructions
            i = 0
            while i < len(insts):
                inst = insts[i]
                si = inst.sync_info
                cap = 2 if isinstance(inst, mybir.InstEventSemaphore) else 1
                if si is not None and len(si.on_wait) > cap:
                    waits = list(si.on_wait)
                    keep, extra = waits[-cap:], waits[:-cap]
                    inst.sync_info = mybir.SyncInfo(
                        on_wait=keep, on_update=list(si.on_update)
                    )
                    for w in extra:
                        n += 1
                        nop = mybir.InstNoOp(name=f"WSPLIT-{n}", ins=[], outs=[])
                        nop.engine = inst.engine
                        nop.sync_info = mybir.SyncInfo(on_wait=[w], on_update=[])
                        insts.insert(i, nop)
                        i += 1
                i += 1
    return n


def _get_nc():
    if "nc" not in _CACHE:
        _CACHE["nc"] = _build_nc()
    return _CACHE["nc"]


def _make_in_maps(x, adj, available_actions, father_weights, W, b):
    x = np.ascontiguousarray(x, dtype=np.float32)
    adj = np.ascontiguousarray(adj, dtype=np.int32)
    av = np.ascontiguousarray(available_actions, dtype=np.float32)
    fw = np.ascontiguousarray(father_weights, dtype=np.float32)
    W = np.ascontiguousarray(W, dtype=np.float32)
    b = np.ascontiguousarray(b, dtype=np.float32)

    # host-side relayout of the structurally-live weight columns (no math)
    wdt = np.ascontiguousarray(
        W[:, :D].T.reshape(DCH, 128, A).transpose(1, 0, 2).reshape(128, DCH * A)
    )
    # wfj[pp, j, a] = W[a, 256 + 16*(j*64+pp)]  (pure relayout, no math)
    wfj = np.ascontiguousarray(
        W[:, D::A].T.reshape(N, N, A).transpose(1, 0, 2).reshape(N, N * A)
    )
    fw_row = np.ascontiguousarray(fw[None, :])
    bias = np.ascontiguousarray(b[None, :])

    in_maps = []
    for i in range(M):
        sl = slice(BL * i, BL * (i + 1))
        in_maps.append({
            "adj": np.ascontiguousarray(adj[sl].reshape(R, N)),
            "x": np.ascontiguousarray(x[sl]),
            "avail": np.ascontiguousarray(av[sl]),
            "fw": fw_row,
            "wfj": wfj,
            "wdt": wdt,
            "bias": bias,
        })
    return in_maps


def _assemble(results):
    actions = np.concatenate([r["actions"] for r in results], axis=0).astype(np.int32)
    alp = np.concatenate([r["alp"] for r in results], axis=0).astype(np.float32)
    father = np.concatenate(
        [r["father"].reshape(BL, NG * A) for r in results], axis=0
    ).astype(np.float32)
    return actions, alp, father


def run_on_device(inputs, trace=False, retries=2, **kw):
    """Compile+run the SPMD bass kernel; returns (outputs_tuple, BassKernelResults)."""
    import time

    from concourse.bass_utils import run_bass_kernel_spmd

    nc = _get_nc()
    in_maps = _make_in_maps(**inputs)
    last_err = None
    for attempt in range(retries + 1):
        try:
            res = run_bass_kernel_spmd(
                nc, in_maps, core_ids=list(range(M)), trace=trace, **kw
            )
            return _assemble(res.results), res
        except Exception as e:  # transient NRT/device errors recover on retry
            msg = str(e)
            if attempt < retries and (
                "UNRECOVERABLE" in msg or "UNAVAILABLE" in msg
                or "PassThrough" in msg
            ):
                last_err = e
                time.sleep(2.0)
                continue
            raise
    raise last_err


def kernel(**inputs):
    (actions, alp, father), _ = run_on_device(inputs, trace=False)
    return actions, alp, father


# revision 51
# speedup vs baseline: 1.0949x; 1.0019x over previous
"""Trainium2 Bass kernel for nn_ACTGraphLayer (gnn_message_passing).

Data-parallel over B=256 rollout threads: 32 rows per NeuronCore x 8 cores.

Per core:
  - father_flat shard [32, 65536] is viewed as [2048, 1024] rows; only every
    16th float (slot 0 of each action block) is nonzero: adj[b,j,p]*fw[p].
    Partition p owns output rows 16p..16p+15 so the adj load and all stores
    are contiguous per partition.  Zero-initialized [128, 1024] SBUF tiles
    get their stride-16 slot-0 lanes overwritten by strided vector copies,
    then stream out as 16 x 512 KB contiguous stores alternating across the
    two HWDGE rings (sync + scalar).
  - logits[b,a] = x@Wd.T + sum_{j,pp} adjf[b,j,pp]*Wf[(j,pp),a] + bias on
    the TensorEngine.  The PE path reuses ajf (fw already folded): for each
    row-group q, T_q = transpose(ajf[:,q,:]) puts parents pp on partitions
    with free index 4b+s, so the strided slice T_q[:, s::4] is exactly
    adjf[b, j=16s+q, :].T -- no second adj read and no extra convert.  64
    K=64 matmuls against j-grouped weight chunks (host relayout only, no
    host arithmetic) + 2 K=128 matmuls for the x part accumulate in PSUM.
    Softmax/argmax run on-device (max8/max_index + Exp/Ln).
"""

import os
import sys

import numpy as np

for _p in ("/opt/trn_rl_repo", "/root/.axon_site/_ro/trn_rl_repo"):
    if os.path.isdir(_p) and _p not in sys.path:
        sys.path.insert(0, _p)

B, N, A, D = 256, 64, 16, 256
M = 8                      # cores
BL = B // M                # 32 rows per core
NG = N * N                 # 4096 father groups per row
R = BL * N                 # 2048 output rows of 1024 floats per core
KCH = NG // 128            # 32 matmul K-chunks over the father axis
DCH = D // 128             # 2 matmul K-chunks over the x axis
NEG_INF = -1e10
TCH = R // 128             # 16 row-groups per partition
SUB = 1                    # row-groups per store (512 KB stores)
NST = TCH // SUB           # 16 stores
NEXP = 10                  # expansion tile ring depth

_CACHE = {}


def _build_nc():
    import concourse.bass as bass
    import concourse.mybir as mybir
    import concourse.tile as tile
    from concourse.masks import make_identity
    from contextlib import ExitStack

    f32 = mybir.dt.float32
    i32 = mybir.dt.int32
    u32 = mybir.dt.uint32

    nc = bass.Bass()
    adj_d = nc.declare_dram_parameter("adj", [R, N], i32, isOutput=False)
    x_d = nc.declare_dram_parameter("x", [BL, D], f32, isOutput=False)
    av_d = nc.declare_dram_parameter("avail", [BL, A], f32, isOutput=False)
    fw_d = nc.declare_dram_parameter("fw", [1, N], f32, isOutput=False)
    wfj_d = nc.declare_dram_parameter("wfj", [N, N * A], f32, False)
    wdt_d = nc.declare_dram_parameter("wdt", [128, DCH * A], f32, isOutput=False)
    b_d = nc.declare_dram_parameter("bias", [1, A], f32, isOutput=False)
    fat_d = nc.declare_dram_parameter("father", [R, N * A], f32, isOutput=True)
    act_d = nc.declare_dram_parameter("actions", [BL, 1], i32, isOutput=True)
    alp_d = nc.declare_dram_parameter("alp", [BL, 1], f32, isOutput=True)

    with ExitStack() as ctx:
        tc = ctx.enter_context(tile.TileContext(nc))
        singles = ctx.enter_context(tc.tile_pool(name="singles", bufs=1))
        expp = ctx.enter_context(tc.tile_pool(name="expp", bufs=1))
        psump = ctx.enter_context(tc.tile_pool(name="psump", bufs=6, space="PSUM"))
        lgp = ctx.enter_context(tc.tile_pool(name="lgp", bufs=1, space="PSUM"))

        # ---- early loads ----
        # sync ring: adj in expansion layout first (feeds the store stream).
        # Partition p holds output rows 16p..16p+15, so the load is fully
        # contiguous per partition (4 KB descriptors) and each store writes
        # 128 contiguous 4 KB runs.
        aj_all = singles.tile([128, TCH, N], i32)
        aj_src = adj_d[:].rearrange("(p t) n -> p t n", p=128)
        PIECES = [(0, 2), (2, 6), (8, 4), (12, 4)]  # (start group, n groups)
        for c, (s0, ln) in enumerate(PIECES):
            eng = nc.sync if c % 2 == 0 else nc.scalar
            eng.dma_start(
                out=aj_all[:, s0:s0 + ln, :], in_=aj_src[:, s0:s0 + ln, :]
            )
        # weight/small loads are emitted later (interleaved into the scalar
        # ring behind its first stores); declare tiles here
        wfj_sb = singles.tile([N, N, A], f32)
        xb = singles.tile([BL, D], f32)
        wd_sb = singles.tile([128, DCH * A], f32)
        av_sb = singles.tile([BL, A], f32)

        # POOL: broadcasts + identity + PE-path cast-load + half the memsets
        fwrow = singles.tile([128, N], f32)
        nc.gpsimd.dma_start(out=fwrow[:], in_=fw_d[:].to_broadcast((128, N)))
        ident = singles.tile([128, 128], f32)
        make_identity(nc, ident[:])
        exp_tiles = [
            expp.tile([128, SUB, N * A], f32, tag=f"exp{i}", name=f"exp{i}")
            for i in range(NEXP)
        ]
        # tiles are zeroed just-in-time inside the store loop (DVE/POOL split)

        b_sb = singles.tile([BL, A], f32)
        nc.gpsimd.dma_start(out=b_sb[:], in_=b_d[:].to_broadcast((BL, A)))

        # ---- expansion stream: ajf piece -> copies -> stores, interleaved ----
        ajf = singles.tile([128, TCH, N], f32)
        fw3 = fwrow[:].rearrange("p (o n) -> p o n", o=1)
        fat_v = fat_d[:].rearrange("(p u c) n -> u p c n", u=NST, c=SUB)
        piece_starts = {s0: ln for s0, ln in PIECES}
        wloads = [(wfj_sb, wfj_d), (xb, x_d), (wd_sb, wdt_d), (av_sb, av_d)]
        for u in range(NST):
            ex = exp_tiles[u % NEXP]
            if u < NEXP:
                (nc.vector if u % 2 == 0 else nc.gpsimd).memset(ex[:], 0.0)
            exv = ex[:].rearrange("p c (g a) -> p c g a", a=A)
            for i in range(SUB):
                t = SUB * u + i
                if t in piece_starts:
                    ln = piece_starts[t]
                    nc.vector.tensor_tensor(
                        out=ajf[:, t:t + ln, :],
                        in0=aj_all[:, t:t + ln, :],
                        in1=fw3.to_broadcast((128, ln, N)),
                        op=mybir.AluOpType.mult,
                    )
                nc.vector.tensor_copy(
                    out=exv[:, i, :, 0:1],
                    in_=ajf[:, t, :].rearrange("p (g o) -> p g o", o=1),
                )
            # ACT ring only takes early stores so it frees up for the
            # epilogue activations; SP takes the rest
            eng = nc.scalar if (u % 2 == 1 and u < 12) else nc.sync
            eng.dma_start(out=fat_v[u], in_=ex[:])
            # slot the weight/small loads onto the scalar ring behind its
            # first stores (they are needed only by the PE/epilogue phase)
            if u == 1:
                for tile_, src_ in wloads[:2]:
                    nc.scalar.dma_start(out=tile_[:], in_=src_[:])
            if u == 3:
                for tile_, src_ in wloads[2:]:
                    nc.scalar.dma_start(out=tile_[:], in_=src_[:])

        # ---- PE path: transpose ajf itself (fw already folded) ----
        # T_q[n, p] = ajf[p, q, n] = adjf[b=p//4, j=16*(p%4)+q, n], so the
        # strided free slice [:, s::4] of T_q is exactly adjf[b, j=16s+q, :].T
        # -- no second adj read and no separate f32 convert needed.
        tsb = singles.tile([N, TCH, 128], f32)
        for q in range(TCH):
            ptq = psump.tile([N, 128], f32, tag="pt", name="ptq")
            nc.tensor.transpose(ptq[:], ajf[:, q, :], ident[:, :])
            nc.vector.tensor_copy(out=tsb[:, q, :], in_=ptq[:])
        xt = singles.tile([128, DCH * BL], f32)
        for c in range(DCH):
            ptx = psump.tile([128, BL], f32, tag="pt", name="ptx")
            nc.tensor.transpose(
                ptx[:], xb[:, 128 * c:128 * (c + 1)], ident[:BL, :BL]
            )
            nc.vector.tensor_copy(
                out=xt[:, c * BL:(c + 1) * BL], in_=ptx[:]
            )

        lg = lgp.tile([BL, A], f32)
        for j in range(N):
            s, q = j // TCH, j % TCH
            lhs = tsb[:, q, :].rearrange("p (b s) -> p s b", s=4)[:, s, :]
            nc.tensor.matmul(
                lg[:], lhsT=lhs, rhs=wfj_sb[:, j, :],
                start=(j == 0), stop=False,
            )
        for c in range(DCH):
            nc.tensor.matmul(
                lg[:],
                lhsT=xt[:, BL * c:BL * (c + 1)],
                rhs=wd_sb[:, A * c:A * (c + 1)],
                start=False,
                stop=(c == DCH - 1),
            )

        # ---- mask + log-softmax + argmax ----
        neg_sb = singles.tile([BL, A], f32)
        nc.vector.memset(neg_sb[:], NEG_INF)
        av_m = singles.tile([BL, A], mybir.dt.int8)
        nc.vector.tensor_scalar(
            out=av_m[:], in0=av_sb[:], scalar1=0.0, scalar2=None,
            op0=mybir.AluOpType.is_gt,
        )
        lg_sb = singles.tile([BL, A], f32)
        nc.vector.tensor_tensor(
            out=lg_sb[:], in0=lg[:], in1=b_sb[:], op=mybir.AluOpType.add
        )
        ml = singles.tile([BL, A], f32)
        nc.vector.tensor_copy(out=ml[:], in_=neg_sb[:])
        nc.vector.copy_predicated(out=ml[:], mask=av_m[:], data=lg_sb[:])

        m8 = singles.tile([BL, 8], f32)
        i8u = singles.tile([BL, 8], u32)
        nc.vector.max(m8[:], ml[:])
        nc.vector.max_index(i8u[:], m8[:], ml[:])

        sh = singles.tile([BL, A], f32)
        nc.vector.tensor_scalar(
            out=sh[:], in0=ml[:], scalar1=m8[:, 0:1], scalar2=None,
            op0=mybir.AluOpType.subtract,
        )
        et = singles.tile([BL, A], f32)
        ssum = singles.tile([BL, 1], f32)
        nc.scalar.activation(
            out=et[:], in_=sh[:], func=mybir.ActivationFunctionType.Exp,
            accum_out=ssum[:],
        )
        lns = singles.tile([BL, 1], f32)
        nc.scalar.activation(
            out=lns[:], in_=ssum[:], func=mybir.ActivationFunctionType.Ln
        )
        alp_sb = singles.tile([BL, 1], f32)
        nc.vector.tensor_scalar_mul(out=alp_sb[:], in0=lns[:], scalar1=-1.0)
        act_sb = singles.tile([BL, 1], i32)
        nc.vector.tensor_copy(out=act_sb[:], in_=i8u[:, 0:1])

        nc.scalar.dma_start(out=act_d[:], in_=act_sb[:])
        nc.scalar.dma_start(out=alp_d[:], in_=alp_sb[:])

    _legalize_multi_waits(nc, mybir)
    return nc


def _legalize_multi_waits(nc, mybir):
    """This toolchain's walrus accepts at most one embedded sync-wait per
    compute/DMA instruction (two for EventSemaphore).  Tile's sem assignment
    can emit more; spill the extras onto same-engine NoOp carriers inserted
    immediately before the instruction."""
    n = 0
    for fn in nc.m.functions:
        for blk in fn.blocks:
            insts = blk.instructions
            i = 0
            while i < len(insts):
                inst = insts[i]
                si = inst.sync_info
                cap = 2 if isinstance(inst, mybir.InstEventSemaphore) else 1
                if si is not None and len(si.on_wait) > cap:
                    waits = list(si.on_wait)
                    keep, extra = waits[-cap:], waits[:-cap]
                    inst.sync_info = mybir.SyncInfo(
                        on_wait=keep, on_update=list(si.on_update)
                    )
                    for w in extra:
                        n += 1
                        nop = mybir.InstNoOp(name=f"WSPLIT-{n}", ins=[], outs=[])
                        nop.engine = inst.engine
                        nop.sync_info = mybir.SyncInfo(on_wait=[w], on_update=[])
                        insts.insert(i, nop)
                        i += 1
                i += 1
    return n


def _get_nc():
    if "nc" not in _CACHE:
        _CACHE["nc"] = _build_nc()
    return _CACHE["nc"]


def _make_in_maps(x, adj, available_actions, father_weights, W, b):
    x = np.ascontiguousarray(x, dtype=np.float32)
    adj = np.ascontiguousarray(adj, dtype=np.int32)
    av = np.ascontiguousarray(available_actions, dtype=np.float32)
    fw = np.ascontiguousarray(father_weights, dtype=np.float32)
    W = np.ascontiguousarray(W, dtype=np.float32)
    b = np.ascontiguousarray(b, dtype=np.float32)

    # host-side relayout of the structurally-live weight columns (no math)
    wdt = np.ascontiguousarray(
        W[:, :D].T.reshape(DCH, 128, A).transpose(1, 0, 2).reshape(128, DCH * A)
    )
    # wfj[pp, j, a] = W[a, 256 + 16*(j*64+pp)]  (pure relayout, no math)
    wfj = np.ascontiguousarray(
        W[:, D::A].T.reshape(N, N, A).transpose(1, 0, 2).reshape(N, N * A)
    )
    fw_row = np.ascontiguousarray(fw[None, :])
    bias = np.ascontiguousarray(b[None, :])

    in_maps = []
    for i in range(M):
        sl = slice(BL * i, BL * (i + 1))
        in_maps.append({
            "adj": np.ascontiguousarray(adj[sl].reshape(R, N)),
            "x": np.ascontiguousarray(x[sl]),
            "avail": np.ascontiguousarray(av[sl]),
            "fw": fw_row,
            "wfj": wfj,
            "wdt": wdt,
            "bias": bias,
        })
    return in_maps


def _assemble(results):
    actions = np.concatenate([r["actions"] for r in results], axis=0).astype(np.int32)
    alp = np.concatenate([r["alp"] for r in results], axis=0).astype(np.float32)
    father = np.concatenate(
        [r["father"].reshape(BL, NG * A) for r in results], axis=0
    ).astype(np.float32)
    return actions, alp, father


def run_on_device(inputs, trace=False, retries=2, **kw):
    """Compile+run the SPMD bass kernel; returns (outputs_tuple, BassKernelResults)."""
    import time

    from concourse.bass_utils import run_bass_kernel_spmd

    nc = _get_nc()
    in_maps = _make_in_maps(**inputs)
    last_err = None
    for attempt in range(retries + 1):
        try:
            res = run_bass_kernel_spmd(
                nc, in_maps, core_ids=list(range(M)), trace=trace, **kw
            )
            return _assemble(res.results), res
        except Exception as e:  # transient NRT/device errors recover on retry
            msg = str(e)
            if attempt < retries and (
                "UNRECOVERABLE" in msg or "UNAVAILABLE" in msg
                or "PassThrough" in msg
            ):
                last_err = e
                time.sleep(2.0)
                continue
            raise
    raise last_err


def kernel(**inputs):
    (actions, alp, father), _ = run_on_device(inputs, trace=False)
    return actions, alp, father


# revision 53
# speedup vs baseline: 1.0984x; 1.0033x over previous
"""Trainium2 Bass kernel for nn_ACTGraphLayer (gnn_message_passing).

Data-parallel over B=256 rollout threads: 32 rows per NeuronCore x 8 cores.

Per core:
  - father_flat shard [32, 65536] is viewed as [2048, 1024] rows; only every
    16th float (slot 0 of each action block) is nonzero: adj[b,j,p]*fw[p].
    Partition p owns output rows 16p..16p+15 so the adj load and all stores
    are contiguous per partition.  Zero-initialized [128, 1024] SBUF tiles
    get their stride-16 slot-0 lanes overwritten by strided vector copies,
    then stream out as 16 x 512 KB contiguous stores alternating across the
    two HWDGE rings (sync + scalar).
  - logits[b,a] = x@Wd.T + sum_{j,pp} adjf[b,j,pp]*Wf[(j,pp),a] + bias on
    the TensorEngine.  The PE path reuses ajf (fw already folded): for each
    row-group q, T_q = transpose(ajf[:,q,:]) puts parents pp on partitions
    with free index 4b+s, so the strided slice T_q[:, s::4] is exactly
    adjf[b, j=16s+q, :].T -- no second adj read and no extra convert.  64
    K=64 matmuls against j-grouped weight chunks (host relayout only, no
    host arithmetic) + 2 K=128 matmuls for the x part accumulate in PSUM.
    Softmax/argmax run on-device (max8/max_index + Exp/Ln).
"""

import os
import sys

import numpy as np

for _p in ("/opt/trn_rl_repo", "/root/.axon_site/_ro/trn_rl_repo"):
    if os.path.isdir(_p) and _p not in sys.path:
        sys.path.insert(0, _p)

B, N, A, D = 256, 64, 16, 256
M = 8                      # cores
BL = B // M                # 32 rows per core
NG = N * N                 # 4096 father groups per row
R = BL * N                 # 2048 output rows of 1024 floats per core
KCH = NG // 128            # 32 matmul K-chunks over the father axis
DCH = D // 128             # 2 matmul K-chunks over the x axis
NEG_INF = -1e10
TCH = R // 128             # 16 row-groups per partition
SUB = 1                    # row-groups per store (512 KB stores)
NST = TCH // SUB           # 16 stores
NEXP = 10                  # expansion tile ring depth

_CACHE = {}


def _build_nc():
    import concourse.bass as bass
    import concourse.mybir as mybir
    import concourse.tile as tile
    from concourse.masks import make_identity
    from contextlib import ExitStack

    f32 = mybir.dt.float32
    i32 = mybir.dt.int32
    u32 = mybir.dt.uint32

    nc = bass.Bass()
    adj_d = nc.declare_dram_parameter("adj", [R, N], i32, isOutput=False)
    x_d = nc.declare_dram_parameter("x", [BL, D], f32, isOutput=False)
    av_d = nc.declare_dram_parameter("avail", [BL, A], f32, isOutput=False)
    fw_d = nc.declare_dram_parameter("fw", [1, N], f32, isOutput=False)
    wfj_d = nc.declare_dram_parameter("wfj", [N, N * A], f32, False)
    wdt_d = nc.declare_dram_parameter("wdt", [128, DCH * A], f32, isOutput=False)
    b_d = nc.declare_dram_parameter("bias", [1, A], f32, isOutput=False)
    fat_d = nc.declare_dram_parameter("father", [R, N * A], f32, isOutput=True)
    act_d = nc.declare_dram_parameter("actions", [BL, 1], i32, isOutput=True)
    alp_d = nc.declare_dram_parameter("alp", [BL, 1], f32, isOutput=True)

    with ExitStack() as ctx:
        tc = ctx.enter_context(tile.TileContext(nc))
        singles = ctx.enter_context(tc.tile_pool(name="singles", bufs=1))
        expp = ctx.enter_context(tc.tile_pool(name="expp", bufs=1))
        psump = ctx.enter_context(tc.tile_pool(name="psump", bufs=6, space="PSUM"))
        lgp = ctx.enter_context(tc.tile_pool(name="lgp", bufs=1, space="PSUM"))

        # ---- early loads ----
        # sync ring: adj in expansion layout first (feeds the store stream).
        # Partition p holds output rows 16p..16p+15, so the load is fully
        # contiguous per partition (4 KB descriptors) and each store writes
        # 128 contiguous 4 KB runs.
        aj_all = singles.tile([128, TCH, N], i32)
        aj_src = adj_d[:].rearrange("(p t) n -> p t n", p=128)
        PIECES = [(0, 2), (2, 6), (8, 4), (12, 4)]  # (start group, n groups)
        for c, (s0, ln) in enumerate(PIECES):
            eng = nc.sync if c % 2 == 0 else nc.scalar
            eng.dma_start(
                out=aj_all[:, s0:s0 + ln, :], in_=aj_src[:, s0:s0 + ln, :]
            )
        # weight/small loads are emitted later (interleaved into the scalar
        # ring behind its first stores); declare tiles here
        wfj_sb = singles.tile([N, N, A], f32)
        xb = singles.tile([BL, D], f32)
        wd_sb = singles.tile([128, DCH * A], f32)
        av_sb = singles.tile([BL, A], f32)

        # POOL: broadcasts + identity + PE-path cast-load + half the memsets
        fwrow = singles.tile([128, N], f32)
        nc.gpsimd.dma_start(out=fwrow[:], in_=fw_d[:].to_broadcast((128, N)))
        ident = singles.tile([128, 128], f32)
        make_identity(nc, ident[:])
        exp_tiles = [
            expp.tile([128, SUB, N * A], f32, tag=f"exp{i}", name=f"exp{i}")
            for i in range(NEXP)
        ]
        # tiles are zeroed just-in-time inside the store loop (DVE/POOL split)

        b_sb = singles.tile([BL, A], f32)
        nc.gpsimd.dma_start(out=b_sb[:], in_=b_d[:].to_broadcast((BL, A)))

        # ---- expansion stream: ajf piece -> copies -> stores, interleaved ----
        ajf = singles.tile([128, TCH, N], f32)
        fw3 = fwrow[:].rearrange("p (o n) -> p o n", o=1)
        fat_v = fat_d[:].rearrange("(p u c) n -> u p c n", u=NST, c=SUB)
        piece_starts = {s0: ln for s0, ln in PIECES}
        wloads = [(wfj_sb, wfj_d), (xb, x_d), (wd_sb, wdt_d), (av_sb, av_d)]
        for u in range(NST):
            ex = exp_tiles[u % NEXP]
            if u < NEXP:
                (nc.vector if u % 2 == 0 else nc.gpsimd).memset(ex[:], 0.0)
            exv = ex[:].rearrange("p c (g a) -> p c g a", a=A)
            for i in range(SUB):
                t = SUB * u + i
                if t in piece_starts:
                    ln = piece_starts[t]
                    nc.vector.tensor_tensor(
                        out=ajf[:, t:t + ln, :],
                        in0=aj_all[:, t:t + ln, :],
                        in1=fw3.to_broadcast((128, ln, N)),
                        op=mybir.AluOpType.mult,
                    )
                nc.vector.tensor_copy(
                    out=exv[:, i, :, 0:1],
                    in_=ajf[:, t, :].rearrange("p (g o) -> p g o", o=1),
                )
            # ACT ring only takes early stores so it frees up for the
            # epilogue activations; SP takes the rest
            eng = nc.scalar if (u % 2 == 1 and u < 12) else nc.sync
            eng.dma_start(out=fat_v[u], in_=ex[:])
            # slot the weight/small loads onto the scalar ring behind its
            # first stores (they are needed only by the PE/epilogue phase)
            if u == 1:
                for tile_, src_ in wloads[:2]:
                    nc.scalar.dma_start(out=tile_[:], in_=src_[:])
            if u == 3:
                for tile_, src_ in wloads[2:]:
                    nc.scalar.dma_start(out=tile_[:], in_=src_[:])

        # ---- PE path: transpose ajf itself (fw already folded) ----
        # T_q[n, p] = ajf[p, q, n] = adjf[b=p//4, j=16*(p%4)+q, n], so the
        # strided free slice [:, s::4] of T_q is exactly adjf[b, j=16s+q, :].T
        # -- no second adj read and no separate f32 convert needed.
        tsb = singles.tile([N, TCH, 128], f32)
        for q in range(TCH):
            ptq = psump.tile([N, 128], f32, tag="pt", name="ptq")
            nc.tensor.transpose(ptq[:], ajf[:, q, :], ident[:, :])
            nc.vector.tensor_copy(out=tsb[:, q, :], in_=ptq[:])
        xt = singles.tile([128, DCH * BL], f32)
        for c in range(DCH):
            ptx = psump.tile([128, BL], f32, tag="pt", name="ptx")
            nc.tensor.transpose(
                ptx[:], xb[:, 128 * c:128 * (c + 1)], ident[:BL, :BL]
            )
            nc.vector.tensor_copy(
                out=xt[:, c * BL:(c + 1) * BL], in_=ptx[:]
            )

        lg = lgp.tile([BL, A], f32)
        for j in range(N):
            s, q = j // TCH, j % TCH
            lhs = tsb[:, q, :].rearrange("p (b s) -> p s b", s=4)[:, s, :]
            nc.tensor.matmul(
                lg[:], lhsT=lhs, rhs=wfj_sb[:, j, :],
                start=(j == 0), stop=False,
            )
        for c in range(DCH):
            nc.tensor.matmul(
                lg[:],
                lhsT=xt[:, BL * c:BL * (c + 1)],
                rhs=wd_sb[:, A * c:A * (c + 1)],
                start=False,
                stop=(c == DCH - 1),
            )

        # ---- mask + log-softmax + argmax ----
        neg_sb = singles.tile([BL, A], f32)
        nc.vector.memset(neg_sb[:], NEG_INF)
        av_m = singles.tile([BL, A], mybir.dt.int8)
        nc.vector.tensor_scalar(
            out=av_m[:], in0=av_sb[:], scalar1=0.0, scalar2=None,
            op0=mybir.AluOpType.is_gt,
        )
        lg_sb = singles.tile([BL, A], f32)
        nc.vector.tensor_tensor(
            out=lg_sb[:], in0=lg[:], in1=b_sb[:], op=mybir.AluOpType.add
        )
        ml = singles.tile([BL, A], f32)
        nc.vector.tensor_copy(out=ml[:], in_=neg_sb[:])
        nc.vector.copy_predicated(out=ml[:], mask=av_m[:], data=lg_sb[:])

        m8 = singles.tile([BL, 8], f32)
        i8u = singles.tile([BL, 8], u32)
        nc.vector.max(m8[:], ml[:])
        nc.vector.max_index(i8u[:], m8[:], ml[:])

        sh = singles.tile([BL, A], f32)
        nc.vector.tensor_scalar(
            out=sh[:], in0=ml[:], scalar1=m8[:, 0:1], scalar2=None,
            op0=mybir.AluOpType.subtract,
        )
        et = singles.tile([BL, A], f32)
        ssum = singles.tile([BL, 1], f32)
        nc.scalar.activation(
            out=et[:], in_=sh[:], func=mybir.ActivationFunctionType.Exp,
            accum_out=ssum[:],
        )
        lns = singles.tile([BL, 1], f32)
        nc.scalar.activation(
            out=lns[:], in_=ssum[:], func=mybir.ActivationFunctionType.Ln
        )
        alp_sb = singles.tile([BL, 1], f32)
        nc.vector.tensor_scalar_mul(out=alp_sb[:], in0=lns[:], scalar1=-1.0)
        act_sb = singles.tile([BL, 1], i32)
        nc.vector.tensor_copy(out=act_sb[:], in_=i8u[:, 0:1])

        nc.scalar.dma_start(out=act_d[:], in_=act_sb[:])
        nc.scalar.dma_start(out=alp_d[:], in_=alp_sb[:])

    _legalize_multi_waits(nc, mybir)
    return nc


def _legalize_multi_waits(nc, mybir):
    """This toolchain's walrus accepts at most one embedded sync-wait per
    compute/DMA instruction (two for EventSemaphore).  Tile's sem assignment
    can emit more; spill the extras onto same-engine NoOp carriers inserted
    immediately before the instruction."""
    n = 0
    for fn in nc.m.functions:
        for blk in fn.blocks:
            insts = blk.instructions
            i = 0
            while i < len(insts):
                inst = insts[i]
                si = inst.sync_info
                cap = 2 if isinstance(inst, mybir.InstEventSemaphore) else 1
                if si is not None and len(si.on_wait) > cap:
                    waits = list(si.on_wait)
                    keep, extra = waits[-cap:], waits[:-cap]
                    inst.sync_info = mybir.SyncInfo(
                        on_wait=keep, on_update=list(si.on_update)
                    )
                    for w in extra:
                        n += 1
                        nop = mybir.InstNoOp(name=f"WSPLIT-{n}", ins=[], outs=[])
                        nop.engine = inst.engine
                        nop.sync_info = mybir.SyncInfo(on_wait=[w], on_update=[])
                        insts.insert(i, nop)
                        i += 1
                i += 1
    return n


def _get_nc():
    if "nc" not in _CACHE:
        _CACHE["nc"] = _build_nc()
    return _CACHE["nc"]


def _make_in_maps(x, adj, available_actions, father_weights, W, b):
    x = np.ascontiguousarray(x, dtype=np.float32)
    adj = np.ascontiguousarray(adj, dtype=np.int32)
    av = np.ascontiguousarray(available_actions, dtype=np.float32)
    fw = np.ascontiguousarray(father_weights, dtype=np.float32)
    W = np.ascontiguousarray(W, dtype=np.float32)
    b = np.ascontiguousarray(b, dtype=np.float32)

    # host-side relayout of the structurally-live weight columns (no math)
    wdt = np.ascontiguousarray(
        W[:, :D].T.reshape(DCH, 128, A).transpose(1, 0, 2).reshape(128, DCH * A)
    )
    # wfj[pp, j, a] = W[a, 256 + 16*(j*64+pp)]  (pure relayout, no math)
    wfj = np.ascontiguousarray(
        W[:, D::A].T.reshape(N, N, A).transpose(1, 0, 2).reshape(N, N * A)
    )
    fw_row = np.ascontiguousarray(fw[None, :])
    bias = np.ascontiguousarray(b[None, :])

    in_maps = []
    for i in range(M):
        sl = slice(BL * i, BL * (i + 1))
        in_maps.append({
            "adj": np.ascontiguousarray(adj[sl].reshape(R, N)),
            "x": np.ascontiguousarray(x[sl]),
            "avail": np.ascontiguousarray(av[sl]),
            "fw": fw_row,
            "wfj": wfj,
            "wdt": wdt,
            "bias": bias,
        })
    return in_maps


def _assemble(results):
    actions = np.concatenate([r["actions"] for r in results], axis=0).astype(np.int32)
    alp = np.concatenate([r["alp"] for r in results], axis=0).astype(np.float32)
    father = np.concatenate(
        [r["father"].reshape(BL, NG * A) for r in results], axis=0
    ).astype(np.float32)
    return actions, alp, father


def run_on_device(inputs, trace=False, retries=2, **kw):
    """Compile+run the SPMD bass kernel; returns (outputs_tuple, BassKernelResults)."""
    import time

    from concourse.bass_utils import run_bass_kernel_spmd

    nc = _get_nc()
    in_maps = _make_in_maps(**inputs)
    last_err = None
    for attempt in range(retries + 1):
        try:
            res = run_bass_kernel_spmd(
                nc, in_maps, core_ids=list(range(M)), trace=trace, **kw
            )
            return _assemble(res.results), res
        except Exception as e:  # transient NRT/device errors recover on retry
            msg = str(e)
            if attempt < retries and (
                "UNRECOVERABLE" in msg or "UNAVAILABLE" in msg
                or "PassThrough" in msg
            ):
                last_err = e
                time.sleep(2.0)
                continue
            raise
    raise last_err


def kernel(**inputs):
    (actions, alp, father), _ = run_on_device(inputs, trace=False)
    return actions, alp, father
